# revision 1
# baseline (speedup 1.0000x reference)
"""BertAttention (B=32, S=512, H=768, 12 heads) Bass/Tile kernel for 8 TRN2 cores.

Sharding: data-parallel over batch — 4 batches per NeuronCore. kernel() takes
the FULL inputs, slices/preps them on host, runs one SPMD NEFF on cores 0-7,
and reassembles the full (32, 512, 768) output.

Per-core pipeline (all matmuls bf16 operands, fp32 PSUM accumulate), fully
interleaved per batch so the PE-dense projections of batch b+1 fill the
ACT-paced attention phase of batch b:
  per batch:
    QT = (Wq x^T + bq)  as [hidden(j), tok]      (KT likewise)
    V  = (x Wv^T + bv)  as [tok, hidden]  with a ones column per head
    per (head pair, key tile):
      scoresT[k,q] = KT^T QT            (contract d=64; head pairs share the
                                         PE array via row groups)
      expT = exp(scoresT/8 + mask[k])   (mask is per-partition -> free via the
                                         ACT bias; no max-shift needed:
                                         |scores/8| is O(5))
    per head:
      wT[d,q] (+ s row) = V^T expT      (contract k; the ones column in V
                                         makes row 64 the softmax sum)
      normalize rows by 1/s (recip -> partition-broadcast DMA via DRAM bounce
      on the idle POOL DGE -> mult)
    attn_out[q,i] = wT^T WoT            (contract hidden)
    y = (x + bo) + attn_out ; LayerNorm over hidden via bn_stats,
    rstd = exp(-0.5 ln(var+eps)) batched per batch so the ACT LUT set only
    swaps twice per batch.

Host folds bo into the residual input and applies ln_w/ln_b on the output.
"""

import sys

for _p in ("/opt/trn_rl_repo",):
    if _p not in sys.path:
        sys.path.insert(0, _p)

import numpy as np
import ml_dtypes

BF16 = ml_dtypes.bfloat16

N_CORES = 8
B_LOC = 4            # batches per core
S = 512              # sequence length
T = B_LOC * S        # tokens per core
H = 768              # hidden
NH = 12              # heads
D = 64               # head size
KT = 6               # 128-wide hidden tiles
TT = T // 128        # 128-wide token tiles (16)
PAIRS = NH // 2      # head pairs == hidden j-tiles (6)
KT4 = S // 128       # 128-wide key-token tiles per batch (4)
VCOL = 130           # V free layout per pair: [d_even(64) | one | d_odd(64) | one]

_CACHE = {}


def _build():
    import concourse.bacc as bacc
    import concourse.tile as tile
    from concourse import mybir

    f32 = mybir.dt.float32
    bf16 = mybir.dt.bfloat16
    AF = mybir.ActivationFunctionType
    OP = mybir.AluOpType

    nc = bacc.Bacc("TRN2", target_bir_lowering=False, debug=False,
                   enable_asserts=False, num_devices=N_CORES)

    xT_d = nc.dram_tensor("xT", [H, T], bf16, kind="ExternalInput").ap()
    xres_d = nc.dram_tensor("xres", [T, H], f32, kind="ExternalInput").ap()
    maskT_d = nc.dram_tensor("maskT", [S, B_LOC], f32, kind="ExternalInput").ap()
    wqT_d = nc.dram_tensor("wqT", [H, H], bf16, kind="ExternalInput").ap()
    wkT_d = nc.dram_tensor("wkT", [H, H], bf16, kind="ExternalInput").ap()
    wvT_d = nc.dram_tensor("wvT", [H, H], bf16, kind="ExternalInput").ap()
    woT_d = nc.dram_tensor("woT", [H, H], bf16, kind="ExternalInput").ap()
    bqt_d = nc.dram_tensor("bqt", [128, KT], f32, kind="ExternalInput").ap()
    bkt_d = nc.dram_tensor("bkt", [128, KT], f32, kind="ExternalInput").ap()
    bv_d = nc.dram_tensor("bv", [H], f32, kind="ExternalInput").ap()
    out_d = nc.dram_tensor("out", [T, H], f32, kind="ExternalOutput").ap()

    import concourse.bass as bass

    xres_t = xres_d.rearrange("(tt p) h -> tt p h", p=128)
    out_t = out_d.rearrange("(tt p) h -> tt p h", p=128)

    with tile.TileContext(nc) as tc:
        with tc.tile_pool(name="persist", bufs=1) as persist, \
             tc.tile_pool(name="qkv", bufs=2) as qkv, \
             tc.tile_pool(name="expp", bufs=7) as expp, \
             tc.tile_pool(name="wtp", bufs=2) as wtp, \
             tc.tile_pool(name="smalls", bufs=4) as smalls, \
             tc.tile_pool(name="wevp", bufs=3) as wevp, \
             tc.tile_pool(name="lnp", bufs=3) as lnp, \
             tc.tile_pool(name="yp", bufs=5) as yp, \
             tc.tile_pool(name="drp", bufs=8, space="DRAM") as drp, \
             tc.tile_pool(name="proj_ps", bufs=2, space="PSUM") as pp, \
             tc.tile_pool(name="sc_ps", bufs=2, space="PSUM") as sc_ps, \
             tc.tile_pool(name="o_ps", bufs=1, space="PSUM") as o_ps:
            # ---- persistent tensors ----
            xT_sb = persist.tile([128, KT, T], bf16)       # [p, kt, tok]
            wq_sb = persist.tile([128, KT, H], bf16)
            wk_sb = persist.tile([128, KT, H], bf16)
            wv_sb = persist.tile([128, KT, H], bf16)
            wo_sb = persist.tile([128, KT, H], bf16)
            bqt_sb = persist.tile([128, KT], f32)
            bkt_sb = persist.tile([128, KT], f32)
            bvb_sb = persist.tile([128, H], f32)           # bv bcast along partitions
            mask_sb = persist.tile([128, KT4, B_LOC], f32)
            eps_sb = persist.tile([128, 1], f32)
            ones64_sb = persist.tile([1, 64], bf16)  # lhsT for PE-side partition bcast

            # input DMAs ordered so batch 0's operands land first
            xT_t = xT_d.rearrange("(kt p) t -> p kt t", p=128)
            nc.sync.dma_start(out=wq_sb, in_=wqT_d.rearrange("(kt p) j -> p kt j", p=128))
            nc.sync.dma_start(out=xT_sb[:, :, 0:S], in_=xT_t[:, :, 0:S])
            nc.sync.dma_start(out=wk_sb, in_=wkT_d.rearrange("(kt p) j -> p kt j", p=128))
            nc.sync.dma_start(out=wv_sb, in_=wvT_d.rearrange("(kt p) j -> p kt j", p=128))
            nc.sync.dma_start(out=bqt_sb, in_=bqt_d)
            nc.sync.dma_start(out=bkt_sb, in_=bkt_d)
            nc.sync.dma_start(
                out=bvb_sb,
                in_=bass.AP(tensor=bv_d.tensor, offset=bv_d.offset,
                            ap=[[0, 128], [1, H]]),
            )
            nc.sync.dma_start(out=mask_sb, in_=maskT_d.rearrange("(kt p) b -> p kt b", p=128))
            for bb in range(1, B_LOC):
                nc.sync.dma_start(out=xT_sb[:, :, bb * S:(bb + 1) * S],
                                  in_=xT_t[:, :, bb * S:(bb + 1) * S])
            nc.sync.dma_start(out=wo_sb, in_=woT_d.rearrange("(jt p) i -> p jt i", p=128))
            nc.vector.memset(eps_sb, 1e-12)
            nc.vector.memset(ones64_sb, 1.0)
            # Pre-load ACT LUT set 6 (natural_log_exp_and_others): it contains
            # every activation this kernel uses (Exp, Identity, Ln), so if the
            # act-table-load pass honors pre-placed loads, all of its
            # per-first-containing-set reload churn (9 loads, ~11.5us ACT,
            # ~2.6us of it serial in the kernel tail) disappears.
            _tables = list(__import__("concourse.hw_specs", fromlist=["x"])
                           .get_activation_tables(nc.m.arch))
            _set6 = _tables.index("natural_log_exp_and_others")
            nc.scalar.add_instruction(mybir.InstLoadActFuncSet(
                name=nc.get_next_instruction_name(), ins=[], outs=[],
                act_func_set_id=_set6))

            bvb_h = bvb_sb.rearrange("p (pr two d) -> p pr two d", two=2, d=64)

            # ---- per-batch emission helpers (software-pipelined below) ----
            def alloc_qkv():
                qb = qkv.tile([128, PAIRS, S], bf16, tag="qb")
                kb = qkv.tile([128, PAIRS, S], bf16, tag="kb")
                vb = qkv.tile([128, KT4, PAIRS, VCOL], bf16, tag="vb")
                vb_pairs = vb.rearrange("p tl pr (two c) -> p tl pr two c", c=65)
                nc.vector.memset(vb_pairs[:, :, :, :, 64:65], 1.0)
                return qb, kb, vb, vb_pairs

            def emit_qk_proj(b, jt, w_sb, b_sb, dst):
                ps = pp.tile([128, S], f32, tag="proj")
                for kt in range(KT):
                    nc.tensor.matmul(
                        ps, w_sb[:, kt, jt * 128:(jt + 1) * 128],
                        xT_sb[:, kt, b * S:(b + 1) * S],
                        start=(kt == 0), stop=(kt == KT - 1))
                nc.scalar.activation(dst[:, jt, :], ps, AF.Identity,
                                     bias=b_sb[:, jt:jt + 1], scale=1.0)

            def emit_v_group(b, vb_pairs, tl, lo_pr, n):
                ps = pp.tile([128, n], f32, tag="proj")
                tt = b * KT4 + tl
                for kt in range(KT):
                    nc.tensor.matmul(
                        ps, xT_sb[:, kt, tt * 128:(tt + 1) * 128],
                        wv_sb[:, kt, lo_pr * 128:lo_pr * 128 + n],
                        start=(kt == 0), stop=(kt == KT - 1))
                ps_h = ps.rearrange("p (pr two d) -> p pr two d", two=2, d=64)
                hi_pr = lo_pr + n // 128
                for two in range(2):
                    nc.vector.tensor_add(
                        vb_pairs[:, tl, lo_pr:hi_pr, two, 0:64],
                        ps_h[:, :, two, :], bvb_h[:, lo_pr:hi_pr, two, :])

            V_GROUPS = [(tl, lo, n) for tl in range(KT4) for lo, n in ((0, 512), (4, 256))]
            # which V groups of the NEXT batch to emit after each pair of the
            # current batch (back-loaded so pair 5's groups cover the gap
            # before the output projection)
            V_SLICE = {0: [0], 1: [1], 2: [2], 3: [3], 4: [4, 5], 5: [6, 7]}

            def emit_proj_slice(b, pr, tiles):
                qb, kb, vb, vb_pairs = tiles
                emit_qk_proj(b, pr, wq_sb, bqt_sb, qb)
                emit_qk_proj(b, pr, wk_sb, bkt_sb, kb)
                for g in V_SLICE[pr]:
                    emit_v_group(b, vb_pairs, *V_GROUPS[g])

            def emit_o_ln(b, wt_sb):
                """Output projection + residual + LN stats for batch b.
                Returns a closure emitting the LN finalize (rstd + normalize
                + output DMAs) — deferred so its two ACT LUT swaps hide
                behind PE work. Stats are emitted after all four residual
                adds so the PSUM o-slot turnaround is only the add."""
                ys = []
                mvb = smalls.tile([128, KT4, 2], f32, tag="mvb")
                for qt in range(KT4):
                    ops = o_ps.tile([128, H], f32, tag="o")
                    for jt in range(KT):
                        lhsT = wt_sb[:, jt, qt * 128:(qt + 1) * 128]
                        nc.tensor.matmul(ops[:, 0:512], lhsT, wo_sb[:, jt, 0:512],
                                         start=(jt == 0), stop=(jt == KT - 1))
                        nc.tensor.matmul(ops[:, 512:H], lhsT, wo_sb[:, jt, 512:H],
                                         start=(jt == 0), stop=(jt == KT - 1))
                    xr = lnp.tile([128, H], f32, tag="xr")
                    nc.sync.dma_start(out=xr, in_=xres_t[b * KT4 + qt])
                    y = yp.tile([128, H], f32, tag="y")
                    nc.vector.tensor_add(y, xr, ops)
                    ys.append(y)
                    stats = smalls.tile([128, 3, 6], f32, tag="st")
                    for g in range(3):
                        nc.vector.bn_stats(stats[:, g, :], y[:, g * 256:(g + 1) * 256])
                    nc.vector.bn_aggr(mvb[:, qt, :], stats)

                def fin():
                    # rstd = exp(-0.5*ln(var+eps)): Ln/Exp keep ACT in two
                    # LUT sets, batched per batch (two swaps per batch)
                    lnv = smalls.tile([128, KT4], f32, tag="lnv")
                    nc.scalar.activation(lnv, mvb[:, :, 1], AF.Ln,
                                         bias=eps_sb, scale=1.0)
                    rstd = smalls.tile([128, KT4], f32, tag="rstd")
                    nc.scalar.activation(rstd, lnv, AF.Exp, bias=0.0, scale=-0.5)
                    for qt in range(KT4):
                        o = lnp.tile([128, H], f32, tag="o")
                        nc.vector.tensor_scalar(o, ys[qt], scalar1=mvb[:, qt, 0:1],
                                                scalar2=rstd[:, qt:qt + 1],
                                                op0=OP.subtract, op1=OP.mult)
                        nc.sync.dma_start(out=out_t[b * KT4 + qt], in_=o)
                return fin

            # prologue: batch 0 projections
            cur = alloc_qkv()
            for pr in range(PAIRS):
                emit_proj_slice(0, pr, cur)

            pending_fin = None
            deferred_o = None
            for b in range(B_LOC):
                qb, kb, vb, _ = cur
                nxt = alloc_qkv() if b + 1 < B_LOC else None

                # ---- attention, interleaved with next batch's projections
                # so the in-order PE stream has projection matmuls to chew on
                # while ACT produces this pair's exp tiles ----
                wt_sb = wtp.tile([128, PAIRS, S], bf16, tag="wt")
                for pr in range(PAIRS):
                    exps = {}
                    for kt in range(KT4):
                        ps = sc_ps.tile([128, 1024], f32, tag="sc")
                        for hh in range(2):
                            lo, hi = hh * 64, (hh + 1) * 64
                            nc.tensor.matmul(
                                ps[:, hh * 512:(hh + 1) * 512],
                                kb[lo:hi, pr, kt * 128:(kt + 1) * 128],
                                qb[lo:hi, pr, :],
                                start=True, stop=True)
                        ex = expp.tile([128, 1024], bf16, tag="ex")
                        nc.scalar.activation(ex, ps, AF.Exp,
                                             bias=mask_sb[:, kt, b:b + 1],
                                             scale=0.125)
                        for hh in range(2):
                            exps[kt, hh] = ex[:, hh * 512:(hh + 1) * 512]
                    if nxt is not None:
                        emit_proj_slice(b + 1, pr, nxt)
                    if pr == 1 and pending_fin is not None:
                        pending_fin()
                        pending_fin = None
                    # both heads' weighted sums; rows 0..63 = sum(attn*V),
                    # row 64 = softmax denominator (ones column of V).
                    # DVE evacuates PSUM to SBUF right away so the PSUM slot
                    # turns around fast (the normalize chain has DMA latency).
                    wev = wevp.tile([65, 1024], f32, tag="wev")
                    for hh in range(2):
                        wps = pp.tile([65, 512], f32, tag="proj")
                        for kt in range(KT4):
                            nc.tensor.matmul(
                                wps, vb[:, kt, pr, hh * 65:(hh + 1) * 65],
                                exps[kt, hh],
                                start=(kt == 0), stop=(kt == KT4 - 1))
                        nc.vector.tensor_copy(out=wev[:, hh * 512:(hh + 1) * 512],
                                              in_=wps)
                    # normalize by 1/s: partition-broadcast of the two recip
                    # rows via a DRAM bounce (SBUF APs cannot have a zero
                    # partition step). Chains alternate between the POOL DGE
                    # and the (mostly idle) HWDGE so consecutive pairs' chains
                    # don't queue behind each other. The very last pair gates
                    # the final output projection with nothing left to hide
                    # the two DRAM round-trips, so it broadcasts on the (then
                    # idle) PE instead: a bf16 outer product ones^T @ (1/s)
                    # into a free scores-pool PSUM slot.
                    dge = nc.sync if pr % 2 else nc.gpsimd
                    if b == B_LOC - 1 and pr == PAIRS - 1:
                        sr = smalls.tile([1, 1024], bf16, tag="srb", bufs=1)
                        with nc.allow_low_precision(reason="bf16 1/s for PE bcast"):
                            nc.vector.reciprocal(sr, wev[64:65, :])
                        bc = sc_ps.tile([64, 1024], f32, tag="sc")
                        for hh in range(2):
                            nc.tensor.matmul(bc[:, hh * 512:(hh + 1) * 512],
                                             ones64_sb,
                                             sr[:, hh * 512:(hh + 1) * 512],
                                             start=True, stop=True)
                    else:
                        sr = smalls.tile([1, 1024], f32, tag="sr", bufs=3)
                        nc.vector.reciprocal(sr, wev[64:65, :])
                        dscr = drp.tile([1, 1024], f32, tag="dscr")
                        dge.dma_start(out=dscr, in_=sr)
                        bc = smalls.tile([64, 1024], f32, tag="bc")
                        dge.dma_start(out=bc, in_=dscr.to_broadcast([64, 1024]))
                    nc.vector.tensor_mul(wt_sb[0:64, pr, :], wev[0:64, 0:512],
                                         bc[:, 0:512])
                    wh = smalls.tile([64, 512], bf16, tag="wh")
                    nc.vector.tensor_mul(wh, wev[0:64, 512:1024], bc[:, 512:1024])
                    dge.dma_start(out=wt_sb[64:128, pr, :], in_=wh)

                if b < B_LOC - 1:
                    pending_fin = emit_o_ln(b, wt_sb)
                else:
                    fin_last = emit_o_ln(b, wt_sb)
                    fin_last()
                cur = nxt

    nc.compile()
    return nc


def _get_nc():
    if "nc" not in _CACHE:
        _CACHE["nc"] = _build()
    return _CACHE["nc"]


def _prep_in_maps(inputs):
    x = np.asarray(inputs["x"], np.float32)
    mask = np.asarray(inputs["additive_attention_mask"], np.float32)
    shared = {
        "wqT": np.ascontiguousarray(np.asarray(inputs["Wq"], np.float32).T).astype(BF16),
        "wkT": np.ascontiguousarray(np.asarray(inputs["Wk"], np.float32).T).astype(BF16),
        "wvT": np.ascontiguousarray(np.asarray(inputs["Wv"], np.float32).T).astype(BF16),
        "woT": np.ascontiguousarray(np.asarray(inputs["Wo"], np.float32).T).astype(BF16),
        "bqt": np.ascontiguousarray(np.asarray(inputs["bq"], np.float32).reshape(KT, 128).T),
        "bkt": np.ascontiguousarray(np.asarray(inputs["bk"], np.float32).reshape(KT, 128).T),
        "bv": np.ascontiguousarray(np.asarray(inputs["bv"], np.float32)),
    }
    bo = np.asarray(inputs["bo"], np.float32)
    in_maps = []
    for c in range(N_CORES):
        xs = x[c * B_LOC:(c + 1) * B_LOC].reshape(T, H)
        in_maps.append({
            "xT": np.ascontiguousarray(xs.T).astype(BF16),
            "xres": np.ascontiguousarray(xs + bo[None, :]),
            "maskT": np.ascontiguousarray(mask[c * B_LOC:(c + 1) * B_LOC, 0, 0, :].T),
            **shared,
        })
    return in_maps


def run(inputs, trace=False):
    """Returns (full_output, BassKernelResults)."""
    from concourse.bass_utils import run_bass_kernel_spmd

    nc = _get_nc()
    in_maps = _prep_in_maps(inputs)
    res = run_bass_kernel_spmd(nc, in_maps, core_ids=list(range(N_CORES)),
                               trace=trace)
    out = np.concatenate(
        [res.results[c]["out"].reshape(B_LOC, S, H) for c in range(N_CORES)], axis=0)
    ln_w = np.asarray(inputs["ln_w"], np.float32)
    ln_b = np.asarray(inputs["ln_b"], np.float32)
    out = out * ln_w[None, None, :] + ln_b[None, None, :]
    return np.ascontiguousarray(out.astype(np.float32)), res


def kernel(**inputs) -> np.ndarray:
    out, _ = run(inputs, trace=False)
    return out



# revision 11
# speedup vs baseline: 1.5359x; 1.5359x over previous
"""BertAttention (B=32, S=512, H=768, 12 heads) Bass/Tile kernel for 8 TRN2 cores.

Sharding: data-parallel over batch — 4 batches per NeuronCore. kernel() takes
the FULL inputs, slices/preps them on host, runs one SPMD NEFF on cores 0-7,
and reassembles the full (32, 512, 768) output.

All matmuls run in fp8 (e4m3); the deep contractions (Q/K/V projections,
attn*V over keys, O projection) use DoubleRow perf mode — two 128-deep
contraction subtiles per instruction at double rate. The scores matmul
contracts only d=64, so it runs as plain fp8 matmuls with the two heads of a
pair sharing the PE array at partition bases 0/64 (matmul operands may only
start at partition 0/32/64, which rules out a 4x32 d-folded DoubleRow):
  - exp() runs on ACT straight out of the scores psum (scale 1/32 folds the
    1/sqrt(64) and the fp8 q/k x2 scales; bias carries mask - ln4 so exp fits
    fp8 range). Softmax denominators are taken TRANSPOSED (s per q-token on
    partitions) by tiny ones-rhs matmuls off the same exp tiles, so the
    reciprocal is partition-parallel ([64,16] per pair), then broadcast to a
    [64, 1024] tile via a DRAM-bounce DMA; one DVE multiply per head both
    evacuates the attn*V psum and normalizes, writing the fp8 O-proj operand.
  - softmax bias bv is folded into bo on host (attn weights sum to 1), the
    residual is pre-scaled by 256 = product of all fp8 scale factors (ln is
    scale-invariant), and ln_w/ln_b are applied host-side on the output.
"""

import sys

for _p in ("/opt/trn_rl_repo",):
    if _p not in sys.path:
        sys.path.insert(0, _p)

import numpy as np
import ml_dtypes

FP8 = ml_dtypes.float8_e4m3
BF16 = ml_dtypes.bfloat16

N_CORES = 8
B_LOC = 4            # batches per core
S = 512              # sequence length
T = B_LOC * S        # tokens per core
H = 768              # hidden
NH = 12              # heads
D = 64               # head size
KT = 6               # 128-wide hidden tiles
PAIRS = NH // 2      # head pairs (6)
KT4 = S // 128       # 128-wide key-token tiles per batch (4)

_CACHE = {}


def _build():
    import concourse.bacc as bacc
    import concourse.tile as tile
    from concourse import mybir
    import concourse.bass as bass

    f32 = mybir.dt.float32
    bf16 = mybir.dt.bfloat16
    fp8 = mybir.dt.float8e4
    AF = mybir.ActivationFunctionType
    OP = mybir.AluOpType
    PM = mybir.MatmulPerfMode

    nc = bacc.Bacc("TRN2", target_bir_lowering=False, debug=False,
                   enable_asserts=False, num_devices=N_CORES)

    xT_d = nc.dram_tensor("xT", [H, T], fp8, kind="ExternalInput").ap()
    xres_d = nc.dram_tensor("xres", [T, H], bf16, kind="ExternalInput").ap()
    maskm_d = nc.dram_tensor("maskm", [S, B_LOC], f32, kind="ExternalInput").ap()
    wq_d = nc.dram_tensor("wq", [H, H], fp8, kind="ExternalInput").ap()
    wk_d = nc.dram_tensor("wk", [H, H], fp8, kind="ExternalInput").ap()
    wv_d = nc.dram_tensor("wv", [H, H], fp8, kind="ExternalInput").ap()
    wo_d = nc.dram_tensor("wo", [D, NH, H], fp8, kind="ExternalInput").ap()
    bq2_d = nc.dram_tensor("bq2", [128, KT], f32, kind="ExternalInput").ap()
    bk2_d = nc.dram_tensor("bk2", [128, KT], f32, kind="ExternalInput").ap()
    out_d = nc.dram_tensor("out", [T, H], f32, kind="ExternalOutput").ap()

    xres_t = xres_d.rearrange("(tt p) h -> tt p h", p=128)
    out_t = out_d.rearrange("(tt p) h -> tt p h", p=128)

    with tile.TileContext(nc) as tc:
        with tc.tile_pool(name="persist", bufs=1) as persist, \
             tc.tile_pool(name="qkv", bufs=2) as qkv, \
             tc.tile_pool(name="expp", bufs=3) as expp, \
             tc.tile_pool(name="wtp", bufs=2) as wtp, \
             tc.tile_pool(name="sbcp", bufs=3) as sbcp, \
             tc.tile_pool(name="smalls", bufs=4) as smalls, \
             tc.tile_pool(name="lnp", bufs=3) as lnp, \
             tc.tile_pool(name="yp", bufs=5) as yp, \
             tc.tile_pool(name="drp", bufs=6, space="DRAM") as drp, \
             tc.tile_pool(name="pp", bufs=2, space="PSUM") as pp, \
             tc.tile_pool(name="scp", bufs=2, space="PSUM") as scp, \
             tc.tile_pool(name="wevp", bufs=2, space="PSUM") as wevp:
            # ---- persistent tensors ----
            xT_sb = persist.tile([128, KT, T], fp8)       # [p, kt, tok]
            wq_sb = persist.tile([128, KT, H], fp8)
            wk_sb = persist.tile([128, KT, H], fp8)
            wv_sb = persist.tile([128, KT, H], fp8)
            wo_sb = persist.tile([D, NH, H], fp8)         # [d, head, hid_out]
            bq2_sb = persist.tile([128, KT], f32)
            bk2_sb = persist.tile([128, KT], f32)
            mask_sb = persist.tile([128, KT4, B_LOC], f32)
            eps_sb = persist.tile([128, 1], f32)
            ones8_sb = persist.tile([128, 2, 1], fp8)     # 0.25: softmax-sum rhs

            xT_t = xT_d.rearrange("(kt p) t -> p kt t", p=128)
            wq_t = wq_d.rearrange("(kt p) j -> p kt j", p=128)
            wk_t = wk_d.rearrange("(kt p) j -> p kt j", p=128)
            wv_t = wv_d.rearrange("(kt p) j -> p kt j", p=128)
            # ordered so batch-0 pair-0 operands land first: tiny tensors,
            # x(b0), then per-jt column chunks of Wq/Wk interleaved with Wv
            nc.sync.dma_start(out=bq2_sb, in_=bq2_d)
            nc.sync.dma_start(out=bk2_sb, in_=bk2_d)
            nc.sync.dma_start(out=mask_sb, in_=maskm_d.rearrange("(kt p) b -> p kt b", p=128))
            nc.sync.dma_start(out=xT_sb[:, :, 0:S], in_=xT_t[:, :, 0:S])
            nc.scalar.dma_start(out=wq_sb, in_=wq_t)
            nc.gpsimd.dma_start(out=wk_sb, in_=wk_t)
            nc.scalar.dma_start(out=wv_sb, in_=wv_t)
            for bb in range(1, B_LOC):
                nc.sync.dma_start(out=xT_sb[:, :, bb * S:(bb + 1) * S],
                                  in_=xT_t[:, :, bb * S:(bb + 1) * S])
            nc.gpsimd.dma_start(out=wo_sb, in_=wo_d)
            nc.vector.memset(eps_sb, 1e-12)
            nc.vector.memset(ones8_sb, 0.25)
            # Pre-load ACT LUT set 6 (natural_log_exp_and_others): holds Exp
            # and Ln, the only ACT functions used, so no table reloads occur.
            _tables = list(__import__("concourse.hw_specs", fromlist=["x"])
                           .get_activation_tables(nc.m.arch))
            _set6 = _tables.index("natural_log_exp_and_others")
            nc.scalar.add_instruction(mybir.InstLoadActFuncSet(
                name=nc.get_next_instruction_name(), ins=[], outs=[],
                act_func_set_id=_set6))

            # ---- per-batch emission helpers (software-pipelined below) ----
            def alloc_qkv():
                qb = qkv.tile([128, PAIRS, S], fp8, tag="qb")  # [p, jt, tok]
                kb = qkv.tile([128, PAIRS, S], fp8, tag="kb")
                vb = qkv.tile([128, KT4, NH, D], fp8, tag="vb")  # [ktok, tl, head, d]
                return qb, kb, vb

            def emit_qk_proj(b, t, w_sb, b_sb, dst, on_act=False):
                """One Q/K psum tile jt=t -> fp8 SBUF with bias. Roughly half
                the evacuations go to ACT (Identity+bias) to balance DVE/ACT."""
                ps = pp.tile([128, S], f32, tag="proj")
                for g in range(3):
                    nc.tensor.matmul(
                        ps, w_sb[:, 2 * g:2 * g + 2, t * 128:(t + 1) * 128],
                        xT_sb[:, 2 * g:2 * g + 2, b * S:(b + 1) * S],
                        start=(g == 0), stop=(g == 2), perf_mode=PM.DoubleRow)
                if on_act:
                    nc.scalar.activation(dst[:, t, :], ps, AF.Identity,
                                         bias=b_sb[:, t:t + 1], scale=2.0 ** -5)
                else:
                    nc.vector.tensor_scalar(out=dst[:, t, :], in0=ps,
                                            scalar1=2.0 ** -5,
                                            scalar2=b_sb[:, t:t + 1],
                                            op0=OP.mult, op1=OP.add)

            def emit_v_group(b, vb, tl, cg):
                n = 512 if cg == 0 else 256
                ps = pp.tile([128, n], f32, tag="proj")
                tt = b * KT4 + tl
                for g in range(3):
                    nc.tensor.matmul(
                        ps, xT_sb[:, 2 * g:2 * g + 2, tt * 128:(tt + 1) * 128],
                        wv_sb[:, 2 * g:2 * g + 2, cg * 512:cg * 512 + n],
                        start=(g == 0), stop=(g == 2), perf_mode=PM.DoubleRow)
                nc.vector.tensor_scalar(out=vb[:, tl, cg * 8:cg * 8 + n // D, :],
                                        in0=ps, scalar1=2.0 ** -6, scalar2=None,
                                        op0=OP.mult)

            V_GROUPS = [(tl, cg) for tl in range(KT4) for cg in range(2)]
            V_SLICE = {0: [0], 1: [1], 2: [2], 3: [3], 4: [4, 5], 5: [6, 7]}

            def emit_proj_slice(b, pr, tiles):
                qb, kb, vb = tiles
                emit_qk_proj(b, pr, wq_sb, bq2_sb, qb)
                emit_qk_proj(b, pr, wk_sb, bk2_sb, kb, on_act=(pr % 2 == 1))
                for g in V_SLICE[pr]:
                    emit_v_group(b, vb, *V_GROUPS[g])

            def emit_o_chunk(ost, qt):
                """O projection + residual + LN stats for one 128-token tile
                of batch ost['b'] (spread across the NEXT batch's pair loop)."""
                b, wt_sb, xrs = ost["b"], ost["wt"], ost["xrs"]
                y = yp.tile([128, H], f32, tag="y")
                for cg in range(2):
                    n = 512 if cg == 0 else 256
                    ops = pp.tile([128, n], f32, tag="proj")
                    for j in range(PAIRS):
                        nc.tensor.matmul(
                            ops, wt_sb[:, 2 * j:2 * j + 2, qt * 128:(qt + 1) * 128],
                            wo_sb[:, 2 * j:2 * j + 2, cg * 512:cg * 512 + n],
                            start=(j == 0), stop=(j == PAIRS - 1),
                            perf_mode=PM.DoubleRow)
                    nc.vector.tensor_add(y[:, cg * 512:cg * 512 + n],
                                         xrs[qt][:, cg * 512:cg * 512 + n], ops)
                ost["ys"].append(y)
                stats = smalls.tile([128, 3, 6], f32, tag="st")
                for g in range(3):
                    nc.vector.bn_stats(stats[:, g, :], y[:, g * 256:(g + 1) * 256])
                nc.vector.bn_aggr(ost["mvb"][:, qt, :], stats)

            def emit_fin(ost, qts, alt_engine=False):
                """LN finalize (rstd + normalize + out DMAs) for tiles qts.
                rstd = exp(-0.5*ln(var+eps)); Ln and Exp share LUT set 6."""
                b, mvb, ys = ost["b"], ost["mvb"], ost["ys"]
                q0, nq = qts[0], len(qts)
                lnv = smalls.tile([128, nq], f32, tag="lnv")
                nc.scalar.activation(lnv, mvb[:, q0:q0 + nq, 1], AF.Ln,
                                     bias=eps_sb, scale=1.0)
                rstd = smalls.tile([128, nq], f32, tag="rstd")
                nc.scalar.activation(rstd, lnv, AF.Exp, bias=0.0, scale=-0.5)
                for i, qt in enumerate(qts):
                    o = lnp.tile([128, H], f32, tag="o")
                    eng = nc.vector if (alt_engine and qt % 2) else nc.gpsimd
                    eng.tensor_scalar(out=o, in0=ys[qt],
                                      scalar1=mvb[:, qt, 0:1],
                                      scalar2=rstd[:, i:i + 1],
                                      op0=OP.subtract, op1=OP.mult)
                    nc.gpsimd.dma_start(out=out_t[b * KT4 + qt], in_=o)

            def emit_scores_exp(b, pr, qb, kb):
                ex = expp.tile([128, KT4, 2 * S], fp8, tag="ex")
                for kt in range(KT4):
                    ps = scp.tile([128, 1024], f32, tag="sc")
                    for hh in range(2):
                        lo, hi = hh * 64, (hh + 1) * 64
                        nc.tensor.matmul(
                            ps[:, hh * 512:(hh + 1) * 512],
                            kb[lo:hi, pr, kt * 128:(kt + 1) * 128],
                            qb[lo:hi, pr, :],
                            start=True, stop=True)
                    nc.scalar.activation(ex[:, kt, :], ps, AF.Exp,
                                         bias=mask_sb[:, kt, b:b + 1],
                                         scale=2.0 ** -5)
                return ex

            def emit_s_chain(ex):
                """Transposed softmax sums sT[q%64, hh*8+qs] = sum_k ex/4 (tiny
                ones-rhs matmuls into the proj psum ring), partition-parallel
                reciprocal, then scatter+broadcast 1/s to a [64, 1024] tile."""
                st = pp.tile([64, 16], f32, tag="proj")
                for hh in range(2):
                    for qs in range(8):
                        c0 = hh * 512 + qs * 64
                        nc.tensor.matmul(
                            st[:, hh * 8 + qs:hh * 8 + qs + 1],
                            ex[:, 0:2, c0:c0 + 64], ones8_sb,
                            start=True, stop=False, perf_mode=PM.DoubleRow)
                        nc.tensor.matmul(
                            st[:, hh * 8 + qs:hh * 8 + qs + 1],
                            ex[:, 2:4, c0:c0 + 64], ones8_sb,
                            start=False, stop=True, perf_mode=PM.DoubleRow)
                str_sb = smalls.tile([64, 16], f32, tag="str", bufs=3)
                nc.vector.reciprocal(str_sb, st)
                # scatter 1/s into final column order dsr[c*64+l], so the
                # broadcast is one clean 2-dim contiguous DMA
                dsr = drp.tile([1, 1024], f32, tag="dsr")
                nc.sync.dma_start(
                    out=bass.AP(tensor=dsr.tensor, offset=dsr.offset,
                                ap=[[1, 64], [64, 16], [1, 1]]),
                    in_=str_sb)
                sbc = sbcp.tile([64, 1024], f32, tag="sbc")
                nc.sync.dma_start(out=sbc, in_=dsr.to_broadcast([64, 1024]))
                return sbc

            def emit_attnv_mult(vb, wt_sb, pr, ex, sbc):
                for hh in range(2):
                    h = 2 * pr + hh
                    wev = wevp.tile([64, 512], f32, tag="wev")
                    for g in range(2):
                        nc.tensor.matmul(
                            wev, vb[:, 2 * g:2 * g + 2, h, :],
                            ex[:, 2 * g:2 * g + 2, hh * 512:(hh + 1) * 512],
                            start=(g == 0), stop=(g == 1),
                            perf_mode=PM.DoubleRow)
                    nc.vector.tensor_mul(wt_sb[:, h, :], wev,
                                         sbc[:, hh * 512:(hh + 1) * 512])

            # prologue: batch 0 projections
            cur = alloc_qkv()
            for pr in range(PAIRS):
                emit_proj_slice(0, pr, cur)

            pending = None   # o_ln state of the previous batch
            prev = None      # (vb, wt, pr, ex, sbc) attnv pipeline carry
            for b in range(B_LOC):
                qb, kb, vb = cur
                nxt = alloc_qkv() if b + 1 < B_LOC else None

                wt_sb = wtp.tile([64, NH, S], fp8, tag="wt")
                xrs = []
                # attn*V + normalize run one pair behind scores/exp (so the
                # 1/s scatter+broadcast DMA chain has a full pair of slack and
                # the DVE stream never head-of-line blocks on it); the
                # PREVIOUS batch's O-projection/LN spreads across pairs 0-4.
                for pr in range(PAIRS):
                    ex = emit_scores_exp(b, pr, qb, kb)
                    if nxt is not None:
                        emit_proj_slice(b + 1, pr, nxt)
                    if prev is not None:
                        emit_attnv_mult(*prev)
                    if pending is not None:
                        if pr < KT4:
                            emit_o_chunk(pending, pr)
                        elif pr == KT4:
                            emit_fin(pending, range(KT4))
                            pending = None
                    if pr == 3:
                        for qt in range(KT4):
                            xr = lnp.tile([128, H], bf16, tag="xr", bufs=8)
                            nc.gpsimd.dma_start(out=xr, in_=xres_t[b * KT4 + qt])
                            xrs.append(xr)
                    prev = (vb, wt_sb, pr, ex, emit_s_chain(ex))

                mvb = smalls.tile([128, KT4, 2], f32, tag="mvb")
                ost = {"b": b, "wt": wt_sb, "xrs": xrs, "ys": [], "mvb": mvb}
                if b < B_LOC - 1:
                    pending = ost
                else:
                    # tail: flush the last pair, then per-qt finalize on
                    # alternating engines to shorten the serial drain
                    emit_attnv_mult(*prev)
                    prev = None
                    for qt in range(KT4):
                        emit_o_chunk(ost, qt)
                        emit_fin(ost, [qt], alt_engine=True)
                cur = nxt

    nc.compile()
    return nc


def _get_nc():
    if "nc" not in _CACHE:
        _CACHE["nc"] = _build()
    return _CACHE["nc"]


def _prep_in_maps(inputs):
    x = np.asarray(inputs["x"], np.float32)
    mask = np.asarray(inputs["additive_attention_mask"], np.float32)
    Wq = np.asarray(inputs["Wq"], np.float32)
    Wk = np.asarray(inputs["Wk"], np.float32)
    Wv = np.asarray(inputs["Wv"], np.float32)
    Wo = np.asarray(inputs["Wo"], np.float32)
    bq = np.asarray(inputs["bq"], np.float32)
    bk = np.asarray(inputs["bk"], np.float32)
    bv = np.asarray(inputs["bv"], np.float32)
    bo = np.asarray(inputs["bo"], np.float32)

    wq8 = np.ascontiguousarray(Wq.T * 64.0).astype(FP8)
    wk8 = np.ascontiguousarray(Wk.T * 64.0).astype(FP8)
    wv8 = np.ascontiguousarray(Wv.T * 64.0).astype(FP8)
    wo8 = np.ascontiguousarray(
        (Wo.T * 64.0).reshape(NH, D, H).transpose(1, 0, 2)).astype(FP8)
    bq2 = np.ascontiguousarray((2.0 * bq).reshape(KT, 128).T)
    bk2 = np.ascontiguousarray((2.0 * bk).reshape(KT, 128).T)
    bo2 = bo + Wo @ bv  # attn weights sum to 1: bv passes through to O-proj

    shared = {"wq": wq8, "wk": wk8, "wv": wv8, "wo": wo8,
              "bq2": bq2, "bk2": bk2}
    in_maps = []
    for c in range(N_CORES):
        xs = x[c * B_LOC:(c + 1) * B_LOC].reshape(T, H)
        in_maps.append({
            "xT": np.ascontiguousarray(xs.T).astype(FP8),
            "xres": (np.ascontiguousarray(xs + bo2[None, :]) * 256.0).astype(BF16),
            "maskm": np.ascontiguousarray(
                mask[c * B_LOC:(c + 1) * B_LOC, 0, 0, :].T - np.log(4.0)),
            **shared,
        })
    return in_maps


def run(inputs, trace=False):
    """Returns (full_output, BassKernelResults)."""
    from concourse.bass_utils import run_bass_kernel_spmd

    nc = _get_nc()
    in_maps = _prep_in_maps(inputs)
    res = run_bass_kernel_spmd(nc, in_maps, core_ids=list(range(N_CORES)),
                               trace=trace)
    out = np.concatenate(
        [res.results[c]["out"].reshape(B_LOC, S, H) for c in range(N_CORES)], axis=0)
    ln_w = np.asarray(inputs["ln_w"], np.float32)
    ln_b = np.asarray(inputs["ln_b"], np.float32)
    out = out * ln_w[None, None, :] + ln_b[None, None, :]
    return np.ascontiguousarray(out.astype(np.float32)), res


def kernel(**inputs) -> np.ndarray:
    out, _ = run(inputs, trace=False)
    return out


# revision 32
# speedup vs baseline: 1.6044x; 1.0446x over previous
"""BertAttention (B=32, S=512, H=768, 12 heads) Bass/Tile kernel for 8 TRN2 cores.

Sharding: data-parallel over batch — 4 batches per NeuronCore. kernel() takes
the FULL inputs, slices/preps them on host, runs one SPMD NEFF on cores 0-7,
and reassembles the full (32, 512, 768) output.

All matmuls run in fp8 (e4m3); the deep contractions (Q/K/V projections,
attn*V over keys, O projection) use DoubleRow perf mode — two 128-deep
contraction subtiles per instruction at double rate. The scores matmul
contracts only d=64, so it runs as plain fp8 matmuls with the two heads of a
pair sharing the PE array at partition bases 0/64 (matmul operands may only
start at partition 0/32/64, which rules out a 4x32 d-folded DoubleRow):
  - exp() runs on ACT straight out of the scores psum (scale 1/32 folds the
    1/sqrt(64) and the fp8 q/k x2 scales; bias carries mask - ln4 so exp fits
    fp8 range). Softmax denominators are taken TRANSPOSED (s per q-token on
    partitions) by tiny ones-rhs matmuls off the same exp tiles, so the
    reciprocal is partition-parallel ([64,16] per pair), then broadcast to a
    [64, 1024] tile via a DRAM-bounce DMA; one DVE multiply per head both
    evacuates the attn*V psum and normalizes, writing the fp8 O-proj operand.
  - softmax bias bv is folded into bo on host (attn weights sum to 1), the
    residual is pre-scaled by 256 = product of all fp8 scale factors (ln is
    scale-invariant), and ln_w/ln_b are applied host-side on the output.
"""

import sys

for _p in ("/opt/trn_rl_repo",):
    if _p not in sys.path:
        sys.path.insert(0, _p)

import numpy as np
import ml_dtypes

FP8 = ml_dtypes.float8_e4m3
BF16 = ml_dtypes.bfloat16

N_CORES = 8
B_LOC = 4            # batches per core
S = 512              # sequence length
T = B_LOC * S        # tokens per core
H = 768              # hidden
NH = 12              # heads
D = 64               # head size
KT = 6               # 128-wide hidden tiles
PAIRS = NH // 2      # head pairs (6)
KT4 = S // 128       # 128-wide key-token tiles per batch (4)

_CACHE = {}


def _build():
    import concourse.bacc as bacc
    import concourse.tile as tile
    from concourse import mybir
    import concourse.bass as bass

    f32 = mybir.dt.float32
    bf16 = mybir.dt.bfloat16
    fp8 = mybir.dt.float8e4
    AF = mybir.ActivationFunctionType
    OP = mybir.AluOpType
    PM = mybir.MatmulPerfMode

    nc = bacc.Bacc("TRN2", target_bir_lowering=False, debug=False,
                   enable_asserts=False, num_devices=N_CORES)

    xT_d = nc.dram_tensor("xT", [H, T], fp8, kind="ExternalInput").ap()
    xres_d = nc.dram_tensor("xres", [T, H], bf16, kind="ExternalInput").ap()
    wq_d = nc.dram_tensor("wq", [H, H], fp8, kind="ExternalInput").ap()
    wk_d = nc.dram_tensor("wk", [H, H], fp8, kind="ExternalInput").ap()
    wv_d = nc.dram_tensor("wv", [H, H], fp8, kind="ExternalInput").ap()
    wo_d = nc.dram_tensor("wo", [D, NH, H], fp8, kind="ExternalInput").ap()
    cst_d = nc.dram_tensor("cst", [128, 2 * KT + KT4 * B_LOC], f32,
                           kind="ExternalInput").ap()
    out_d = nc.dram_tensor("out", [T, H], f32, kind="ExternalOutput").ap()

    xres_t = xres_d.rearrange("(tt p) h -> tt p h", p=128)
    out_t = out_d.rearrange("(tt p) h -> tt p h", p=128)

    with tile.TileContext(nc) as tc:
        with tc.tile_pool(name="persist", bufs=1) as persist, \
             tc.tile_pool(name="qkv", bufs=2) as qkv, \
             tc.tile_pool(name="expp", bufs=3) as expp, \
             tc.tile_pool(name="wtp", bufs=2) as wtp, \
             tc.tile_pool(name="sbcp", bufs=3) as sbcp, \
             tc.tile_pool(name="smalls", bufs=4) as smalls, \
             tc.tile_pool(name="lnp", bufs=3) as lnp, \
             tc.tile_pool(name="yp", bufs=5) as yp, \
             tc.tile_pool(name="drp", bufs=6, space="DRAM") as drp, \
             tc.tile_pool(name="pp", bufs=2, space="PSUM") as pp, \
             tc.tile_pool(name="scp", bufs=2, space="PSUM") as scp, \
             tc.tile_pool(name="wevp", bufs=2, space="PSUM") as wevp:
            # ---- persistent tensors ----
            xT_sb = persist.tile([128, KT, T], fp8)       # [p, kt, tok]
            wq_sb = persist.tile([128, KT, H], fp8)
            wk_sb = persist.tile([128, KT, H], fp8)
            wv_sb = persist.tile([128, KT, H], fp8)
            wo_sb = persist.tile([D, NH, H], fp8)         # [d, head, hid_out]
            cst_sb = persist.tile([128, 2 * KT + KT4 * B_LOC], f32)
            bq2_sb = cst_sb[:, 0:KT]
            bk2_sb = cst_sb[:, KT:2 * KT]
            mask_sb = cst_sb[:, 2 * KT:].rearrange("p (kt b) -> p kt b", b=B_LOC)
            eps_sb = persist.tile([128, 1], f32)
            ones8_sb = persist.tile([128, 2, 1], fp8)     # 0.25: softmax-sum rhs

            xT_t = xT_d.rearrange("(kt p) t -> p kt t", p=128)
            wq_t = wq_d.rearrange("(kt p) j -> p kt j", p=128)
            wk_t = wk_d.rearrange("(kt p) j -> p kt j", p=128)
            wv_t = wv_d.rearrange("(kt p) j -> p kt j", p=128)
            # ordered so batch-0 pair-0 operands land first: tiny tensors,
            # x(b0), then per-jt column chunks of Wq/Wk interleaved with Wv
            nc.sync.dma_start(out=cst_sb, in_=cst_d)
            nc.sync.dma_start(out=xT_sb[:, :, 0:S], in_=xT_t[:, :, 0:S])
            for jt in range(PAIRS):
                cs = slice(jt * 128, (jt + 1) * 128)
                nc.sync.dma_start(out=wq_sb[:, :, cs], in_=wq_t[:, :, cs])
                nc.sync.dma_start(out=wk_sb[:, :, cs], in_=wk_t[:, :, cs])
                if jt < 2:
                    cv = slice(jt * 512, jt * 512 + (512 if jt == 0 else 256))
                    nc.sync.dma_start(out=wv_sb[:, :, cv], in_=wv_t[:, :, cv])
            for bb in range(1, B_LOC):
                nc.sync.dma_start(out=xT_sb[:, :, bb * S:(bb + 1) * S],
                                  in_=xT_t[:, :, bb * S:(bb + 1) * S])
            nc.gpsimd.dma_start(out=wo_sb, in_=wo_d)
            nc.vector.memset(eps_sb, 1e-12)
            nc.vector.memset(ones8_sb, 0.25)
            # Pre-load ACT LUT set 6 (natural_log_exp_and_others): holds Exp
            # and Ln, the only ACT functions used, so no table reloads occur.
            _tables = list(__import__("concourse.hw_specs", fromlist=["x"])
                           .get_activation_tables(nc.m.arch))
            _set6 = _tables.index("natural_log_exp_and_others")
            nc.scalar.add_instruction(mybir.InstLoadActFuncSet(
                name=nc.get_next_instruction_name(), ins=[], outs=[],
                act_func_set_id=_set6))

            # ---- per-batch emission helpers (software-pipelined below) ----
            def alloc_qkv():
                qb = qkv.tile([128, PAIRS, S], fp8, tag="qb")  # [p, jt, tok]
                kb = qkv.tile([128, PAIRS, S], fp8, tag="kb")
                vb = qkv.tile([128, KT4, NH, D], fp8, tag="vb")  # [ktok, tl, head, d]
                return qb, kb, vb

            def emit_qk_proj(b, t, w_sb, b_sb, dst, on_act=False):
                """One Q/K psum tile jt=t -> fp8 SBUF with bias. Roughly half
                the evacuations go to ACT (Identity+bias) to balance DVE/ACT."""
                ps = pp.tile([128, S], f32, tag="proj")
                for g in range(3):
                    nc.tensor.matmul(
                        ps, w_sb[:, 2 * g:2 * g + 2, t * 128:(t + 1) * 128],
                        xT_sb[:, 2 * g:2 * g + 2, b * S:(b + 1) * S],
                        start=(g == 0), stop=(g == 2), perf_mode=PM.DoubleRow)
                if on_act:
                    nc.scalar.activation(dst[:, t, :], ps, AF.Identity,
                                         bias=b_sb[:, t:t + 1], scale=2.0 ** -5)
                else:
                    nc.vector.tensor_scalar(out=dst[:, t, :], in0=ps,
                                            scalar1=2.0 ** -5,
                                            scalar2=b_sb[:, t:t + 1],
                                            op0=OP.mult, op1=OP.add)

            def emit_v_group(b, vb, tl, cg):
                n = 512 if cg == 0 else 256
                ps = pp.tile([128, n], f32, tag="proj")
                tt = b * KT4 + tl
                for g in range(3):
                    nc.tensor.matmul(
                        ps, xT_sb[:, 2 * g:2 * g + 2, tt * 128:(tt + 1) * 128],
                        wv_sb[:, 2 * g:2 * g + 2, cg * 512:cg * 512 + n],
                        start=(g == 0), stop=(g == 2), perf_mode=PM.DoubleRow)
                nc.vector.tensor_scalar(out=vb[:, tl, cg * 8:cg * 8 + n // D, :],
                                        in0=ps, scalar1=2.0 ** -6, scalar2=None,
                                        op0=OP.mult)

            V_GROUPS = [(tl, cg) for tl in range(KT4) for cg in range(2)]
            V_SLICE = {0: [0], 1: [1], 2: [2], 3: [3], 4: [4, 5], 5: [6, 7]}

            def emit_proj_slice(b, pr, tiles):
                qb, kb, vb = tiles
                emit_qk_proj(b, pr, wq_sb, bq2_sb, qb)
                emit_qk_proj(b, pr, wk_sb, bk2_sb, kb, on_act=(pr % 2 == 1))
                for g in V_SLICE[pr]:
                    emit_v_group(b, vb, *V_GROUPS[g])

            def emit_o_chunk(ost, qt):
                """O projection + residual + LN stats for one 128-token tile
                of batch ost['b'] (spread across the NEXT batch's pair loop)."""
                b, wt_sb, xrs = ost["b"], ost["wt"], ost["xrs"]
                y = yp.tile([128, H], f32, tag="y")
                # O psum lives in the wev ring (not pp): pp stays 4-deep per
                # pair (Q/K/V/st) so its 2 slots never gate the ACT evacs
                ops = wevp.tile([128, H], f32, tag="wev", bufs=1)
                for cg in range(2):
                    n = 512 if cg == 0 else 256
                    for j in range(PAIRS):
                        nc.tensor.matmul(
                            ops[:, cg * 512:cg * 512 + n],
                            wt_sb[:, 2 * j:2 * j + 2, qt * 128:(qt + 1) * 128],
                            wo_sb[:, 2 * j:2 * j + 2, cg * 512:cg * 512 + n],
                            start=(j == 0), stop=(j == PAIRS - 1),
                            perf_mode=PM.DoubleRow)
                nc.vector.tensor_add(y, xrs[qt], ops)
                ost["ys"].append(y)
                stats = smalls.tile([128, 3, 6], f32, tag="st")
                for g in range(3):
                    nc.vector.bn_stats(stats[:, g, :], y[:, g * 256:(g + 1) * 256])
                nc.vector.bn_aggr(ost["mvb"][:, qt, :], stats)

            def emit_fin(ost, qts, alt_engine=False):
                """LN finalize (rstd + normalize + out DMAs) for tiles qts.
                rstd = exp(-0.5*ln(var+eps)); Ln and Exp share LUT set 6."""
                b, mvb, ys = ost["b"], ost["mvb"], ost["ys"]
                q0, nq = qts[0], len(qts)
                lnv = smalls.tile([128, nq], f32, tag="lnv")
                nc.scalar.activation(lnv, mvb[:, q0:q0 + nq, 1], AF.Ln,
                                     bias=eps_sb, scale=1.0)
                rstd = smalls.tile([128, nq], f32, tag="rstd")
                nc.scalar.activation(rstd, lnv, AF.Exp, bias=0.0, scale=-0.5)
                for i, qt in enumerate(qts):
                    o = lnp.tile([128, H], f32, tag="o")
                    eng = nc.vector if (alt_engine and qt % 2) else nc.gpsimd
                    eng.tensor_scalar(out=o, in0=ys[qt],
                                      scalar1=mvb[:, qt, 0:1],
                                      scalar2=rstd[:, i:i + 1],
                                      op0=OP.subtract, op1=OP.mult)
                    dge = nc.sync if alt_engine else nc.gpsimd
                    dge.dma_start(out=out_t[b * KT4 + qt], in_=o)

            def emit_scores_exp(b, pr, qb, kb):
                ex = expp.tile([128, KT4, 2 * S], fp8, tag="ex")
                for kt in range(KT4):
                    ps = scp.tile([128, 1024], f32, tag="sc")
                    for hh in range(2):
                        lo, hi = hh * 64, (hh + 1) * 64
                        nc.tensor.matmul(
                            ps[:, hh * 512:(hh + 1) * 512],
                            kb[lo:hi, pr, kt * 128:(kt + 1) * 128],
                            qb[lo:hi, pr, :],
                            start=True, stop=True)
                    nc.scalar.activation(ex[:, kt, :], ps, AF.Exp,
                                         bias=mask_sb[:, kt, b:b + 1],
                                         scale=2.0 ** -5)
                return ex

            def emit_s_chain(ex):
                """Transposed softmax sums sT[q%64, hh*8+qs] = sum_k ex/4 (tiny
                ones-rhs matmuls into the proj psum ring), partition-parallel
                reciprocal, then scatter+broadcast 1/s to a [64, 1024] tile."""
                st = pp.tile([64, 16], f32, tag="proj")
                for hh in range(2):
                    for qs in range(8):
                        c0 = hh * 512 + qs * 64
                        nc.tensor.matmul(
                            st[:, hh * 8 + qs:hh * 8 + qs + 1],
                            ex[:, 0:2, c0:c0 + 64], ones8_sb,
                            start=True, stop=False, perf_mode=PM.DoubleRow)
                        nc.tensor.matmul(
                            st[:, hh * 8 + qs:hh * 8 + qs + 1],
                            ex[:, 2:4, c0:c0 + 64], ones8_sb,
                            start=False, stop=True, perf_mode=PM.DoubleRow)
                str_sb = smalls.tile([64, 16], f32, tag="str", bufs=3)
                nc.vector.reciprocal(str_sb, st)
                # scatter 1/s into final column order dsr[c*64+l], so the
                # broadcast is one clean 2-dim contiguous DMA
                dsr = drp.tile([1, 1024], f32, tag="dsr")
                nc.sync.dma_start(
                    out=bass.AP(tensor=dsr.tensor, offset=dsr.offset,
                                ap=[[1, 64], [64, 16], [1, 1]]),
                    in_=str_sb)
                sbc = sbcp.tile([64, 1024], f32, tag="sbc")
                nc.sync.dma_start(out=sbc, in_=dsr.to_broadcast([64, 1024]))
                return sbc

            def emit_attnv_mult(vb, wt_sb, pr, ex, sbc):
                # both heads into one [64, 1024] psum; single-buffered is fine
                # because attnv(p+1) is emitted a full pair after mult(p)
                wev = wevp.tile([64, 1024], f32, tag="wev", bufs=1)
                for hh in range(2):
                    h = 2 * pr + hh
                    for g in range(2):
                        nc.tensor.matmul(
                            wev[:, hh * 512:(hh + 1) * 512],
                            vb[:, 2 * g:2 * g + 2, h, :],
                            ex[:, 2 * g:2 * g + 2, hh * 512:(hh + 1) * 512],
                            start=(g == 0), stop=(g == 1),
                            perf_mode=PM.DoubleRow)
                nc.vector.tensor_mul(wt_sb[:, 2 * pr:2 * pr + 2, :], wev, sbc)

            # prologue: batch 0 projections
            cur = alloc_qkv()
            for pr in range(PAIRS):
                emit_proj_slice(0, pr, cur)

            pending = None   # o_ln state of the previous batch
            st1 = None       # (vb, wt, pr, ex): awaiting s_chain (1 pair back)
            st2 = None       # (vb, wt, pr, ex, sbc): awaiting attnv (2 back)
            for b in range(B_LOC):
                qb, kb, vb = cur
                nxt = alloc_qkv() if b + 1 < B_LOC else None

                wt_sb = wtp.tile([64, NH, S], fp8, tag="wt")
                xrs = []
                # attn*V + normalize run one pair behind scores/exp (so the
                # 1/s scatter+broadcast DMA chain has a full pair of slack and
                # the DVE stream never head-of-line blocks on it); the
                # PREVIOUS batch's O-projection/LN spreads across pairs 0-4.
                for pr in range(PAIRS):
                    ex = emit_scores_exp(b, pr, qb, kb)
                    if nxt is not None:
                        emit_proj_slice(b + 1, pr, nxt)
                    if st2 is not None:
                        emit_attnv_mult(*st2)
                    if pending is not None:
                        if pr < KT4:
                            emit_o_chunk(pending, pr)
                        elif pr == KT4:
                            emit_fin(pending, range(KT4))
                            pending = None
                    if pr == 3:
                        for qt in range(KT4):
                            xr = lnp.tile([128, H], bf16, tag="xr", bufs=8)
                            nc.gpsimd.dma_start(out=xr, in_=xres_t[b * KT4 + qt])
                            xrs.append(xr)
                    st2 = (vb, wt_sb, pr, ex, emit_s_chain(ex))

                mvb = smalls.tile([128, KT4, 2], f32, tag="mvb")
                ost = {"b": b, "wt": wt_sb, "xrs": xrs, "ys": [], "mvb": mvb}
                if b < B_LOC - 1:
                    pending = ost
                else:
                    # tail: flush the in-flight pair, then per-qt finalize on
                    # alternating engines to shorten the drain
                    emit_attnv_mult(*st2)
                    st2 = None
                    for qt in range(KT4):
                        emit_o_chunk(ost, qt)
                        emit_fin(ost, [qt], alt_engine=True)
                cur = nxt

    nc.compile()
    return nc


def _get_nc():
    if "nc" not in _CACHE:
        _CACHE["nc"] = _build()
    return _CACHE["nc"]


def _prep_in_maps(inputs):
    x = np.asarray(inputs["x"], np.float32)
    mask = np.asarray(inputs["additive_attention_mask"], np.float32)
    Wq = np.asarray(inputs["Wq"], np.float32)
    Wk = np.asarray(inputs["Wk"], np.float32)
    Wv = np.asarray(inputs["Wv"], np.float32)
    Wo = np.asarray(inputs["Wo"], np.float32)
    bq = np.asarray(inputs["bq"], np.float32)
    bk = np.asarray(inputs["bk"], np.float32)
    bv = np.asarray(inputs["bv"], np.float32)
    bo = np.asarray(inputs["bo"], np.float32)

    wq8 = np.ascontiguousarray(Wq.T * 64.0).astype(FP8)
    wk8 = np.ascontiguousarray(Wk.T * 64.0).astype(FP8)
    wv8 = np.ascontiguousarray(Wv.T * 64.0).astype(FP8)
    wo8 = np.ascontiguousarray(
        (Wo.T * 64.0).reshape(NH, D, H).transpose(1, 0, 2)).astype(FP8)
    bq2 = (2.0 * bq).reshape(KT, 128).T
    bk2 = (2.0 * bk).reshape(KT, 128).T
    bo2 = bo + Wo @ bv  # attn weights sum to 1: bv passes through to O-proj

    shared = {"wq": wq8, "wk": wk8, "wv": wv8, "wo": wo8}
    in_maps = []
    for c in range(N_CORES):
        xs = x[c * B_LOC:(c + 1) * B_LOC].reshape(T, H)
        # mask as [128, kt, b]: token k = kt*128+p of batch b, minus ln4
        mc = (mask[c * B_LOC:(c + 1) * B_LOC, 0, 0, :] - np.log(4.0))
        mkb = mc.reshape(B_LOC, KT4, 128).transpose(2, 1, 0).reshape(128, KT4 * B_LOC)
        cst = np.concatenate([bq2, bk2, mkb], axis=1).astype(np.float32)
        in_maps.append({
            "xT": np.ascontiguousarray(xs.T).astype(FP8),
            "xres": (np.ascontiguousarray(xs + bo2[None, :]) * 256.0).astype(BF16),
            "cst": np.ascontiguousarray(cst),
            **shared,
        })
    return in_maps


def run(inputs, trace=False):
    """Returns (full_output, BassKernelResults)."""
    from concourse.bass_utils import run_bass_kernel_spmd

    nc = _get_nc()
    in_maps = _prep_in_maps(inputs)
    res = run_bass_kernel_spmd(nc, in_maps, core_ids=list(range(N_CORES)),
                               trace=trace)
    out = np.concatenate(
        [res.results[c]["out"].reshape(B_LOC, S, H) for c in range(N_CORES)], axis=0)
    ln_w = np.asarray(inputs["ln_w"], np.float32)
    ln_b = np.asarray(inputs["ln_b"], np.float32)
    out = out * ln_w[None, None, :] + ln_b[None, None, :]
    return np.ascontiguousarray(out.astype(np.float32)), res


def kernel(**inputs) -> np.ndarray:
    out, _ = run(inputs, trace=False)
    return out


# revision 46
# speedup vs baseline: 1.6791x; 1.0466x over previous
"""BertAttention (B=32, S=512, H=768, 12 heads) Bass/Tile kernel for 8 TRN2 cores.

Sharding: data-parallel over batch — 4 batches per NeuronCore. kernel() takes
the FULL inputs, slices/preps them on host, runs one SPMD NEFF on cores 0-7,
and reassembles the full (32, 512, 768) output.

All matmuls run in fp8 (e4m3); the deep contractions (Q/K/V projections,
attn*V over keys, O projection) use DoubleRow perf mode — two 128-deep
contraction subtiles per instruction at double rate. The scores matmul
contracts only d=64, so it runs as plain fp8 matmuls with the two heads of a
pair sharing the PE array at partition bases 0/64 (matmul operands may only
start at partition 0/32/64, which rules out a 4x32 d-folded DoubleRow):
  - exp() runs on ACT straight out of the scores psum (scale 1/32 folds the
    1/sqrt(64) and the fp8 q/k x2 scales; bias carries mask - ln4 so exp fits
    fp8 range). Softmax denominators are taken TRANSPOSED (s per q-token on
    partitions) by tiny ones-rhs matmuls off the same exp tiles, so the
    reciprocal is partition-parallel ([64,16] per pair), then broadcast to a
    [64, 1024] tile via a DRAM-bounce DMA; one DVE multiply per head both
    evacuates the attn*V psum and normalizes, writing the fp8 O-proj operand.
  - softmax bias bv is folded into bo on host (attn weights sum to 1), the
    residual is pre-scaled by 256 = product of all fp8 scale factors (ln is
    scale-invariant), and ln_w/ln_b are applied host-side on the output.
"""

import sys

for _p in ("/opt/trn_rl_repo",):
    if _p not in sys.path:
        sys.path.insert(0, _p)

import numpy as np
import ml_dtypes

FP8 = ml_dtypes.float8_e4m3
BF16 = ml_dtypes.bfloat16

N_CORES = 8
B_LOC = 4            # batches per core
S = 512              # sequence length
T = B_LOC * S        # tokens per core
H = 768              # hidden
NH = 12              # heads
D = 64               # head size
KT = 6               # 128-wide hidden tiles
PAIRS = NH // 2      # head pairs (6)
KT4 = S // 128       # 128-wide key-token tiles per batch (4)

_CACHE = {}


def _build():
    import concourse.bacc as bacc
    import concourse.tile as tile
    from concourse import mybir
    import concourse.bass as bass

    f32 = mybir.dt.float32
    bf16 = mybir.dt.bfloat16
    fp8 = mybir.dt.float8e4
    AF = mybir.ActivationFunctionType
    OP = mybir.AluOpType
    PM = mybir.MatmulPerfMode

    nc = bacc.Bacc("TRN2", target_bir_lowering=False, debug=False,
                   enable_asserts=False, num_devices=N_CORES)

    xT_d = nc.dram_tensor("xT", [H, T], fp8, kind="ExternalInput").ap()
    xres_d = nc.dram_tensor("xres", [T, H], bf16, kind="ExternalInput").ap()
    wq_d = nc.dram_tensor("wq", [H, H], fp8, kind="ExternalInput").ap()
    wk_d = nc.dram_tensor("wk", [H, H], fp8, kind="ExternalInput").ap()
    wv_d = nc.dram_tensor("wv", [H, H], fp8, kind="ExternalInput").ap()
    wo_d = nc.dram_tensor("wo", [D, NH, H], fp8, kind="ExternalInput").ap()
    cst_d = nc.dram_tensor("cst", [128, 2 * KT + KT4 * B_LOC], f32,
                           kind="ExternalInput").ap()
    out_d = nc.dram_tensor("out", [T, H], f32, kind="ExternalOutput").ap()

    xres_t = xres_d.rearrange("(tt p) h -> tt p h", p=128)
    out_t = out_d.rearrange("(tt p) h -> tt p h", p=128)

    with tile.TileContext(nc) as tc:
        with tc.tile_pool(name="persist", bufs=1) as persist, \
             tc.tile_pool(name="qkv", bufs=2) as qkv, \
             tc.tile_pool(name="expp", bufs=5) as expp, \
             tc.tile_pool(name="wtp", bufs=2) as wtp, \
             tc.tile_pool(name="sbcp", bufs=6) as sbcp, \
             tc.tile_pool(name="smalls", bufs=4) as smalls, \
             tc.tile_pool(name="lnp", bufs=3) as lnp, \
             tc.tile_pool(name="yp", bufs=6) as yp, \
             tc.tile_pool(name="drp", bufs=12, space="DRAM") as drp, \
             tc.tile_pool(name="pp", bufs=2, space="PSUM") as pp, \
             tc.tile_pool(name="scp", bufs=2, space="PSUM") as scp, \
             tc.tile_pool(name="wevp", bufs=2, space="PSUM") as wevp:
            # ---- persistent tensors ----
            xT_sb = persist.tile([128, KT, T], fp8)       # [p, kt, tok]
            wq_sb = persist.tile([128, KT, H], fp8)
            wk_sb = persist.tile([128, KT, H], fp8)
            wv_sb = persist.tile([128, KT, H], fp8)
            wo_sb = persist.tile([D, NH, H], fp8)         # [d, head, hid_out]
            cst_sb = persist.tile([128, 2 * KT + KT4 * B_LOC], f32)
            bq2_sb = cst_sb[:, 0:KT]
            bk2_sb = cst_sb[:, KT:2 * KT]
            mask_sb = cst_sb[:, 2 * KT:].rearrange("p (kt b) -> p kt b", b=B_LOC)
            eps_sb = persist.tile([128, 1], f32)
            ones8_sb = persist.tile([128, 2, 1], fp8)     # 0.25: softmax-sum rhs

            xT_t = xT_d.rearrange("(kt p) t -> p kt t", p=128)
            wq_t = wq_d.rearrange("(kt p) j -> p kt j", p=128)
            wk_t = wk_d.rearrange("(kt p) j -> p kt j", p=128)
            wv_t = wv_d.rearrange("(kt p) j -> p kt j", p=128)
            # ordered so batch-0 pair-0 operands land first: tiny tensors,
            # x(b0), then per-jt column chunks of Wq/Wk interleaved with Wv
            nc.sync.dma_start(out=cst_sb, in_=cst_d)
            nc.sync.dma_start(out=xT_sb[:, :, 0:S], in_=xT_t[:, :, 0:S])
            nc.sync.dma_start(out=wq_sb, in_=wq_t)
            nc.sync.dma_start(out=wk_sb, in_=wk_t)
            nc.sync.dma_start(out=wv_sb, in_=wv_t)
            for bb in range(1, B_LOC):
                nc.sync.dma_start(out=xT_sb[:, :, bb * S:(bb + 1) * S],
                                  in_=xT_t[:, :, bb * S:(bb + 1) * S])
            nc.sync.dma_start(out=wo_sb, in_=wo_d)
            nc.vector.memset(eps_sb, 1e-12)
            nc.vector.memset(ones8_sb, 0.25)
            # Pre-load ACT LUT set 6 (natural_log_exp_and_others): holds Exp
            # and Ln, the only ACT functions used, so no table reloads occur.
            _tables = list(__import__("concourse.hw_specs", fromlist=["x"])
                           .get_activation_tables(nc.m.arch))
            _set6 = _tables.index("natural_log_exp_and_others")
            nc.scalar.add_instruction(mybir.InstLoadActFuncSet(
                name=nc.get_next_instruction_name(), ins=[], outs=[],
                act_func_set_id=_set6))

            # ---- per-batch emission helpers (software-pipelined below) ----
            def alloc_qkv():
                qb = qkv.tile([128, PAIRS, S], fp8, tag="qb")  # [p, jt, tok]
                kb = qkv.tile([128, PAIRS, S], fp8, tag="kb")
                vb = qkv.tile([128, KT4, NH, D], fp8, tag="vb")  # [ktok, tl, head, d]
                return qb, kb, vb

            def emit_qk_proj(b, t, w_sb, b_sb, dst, on_act=False):
                """One Q/K psum tile jt=t -> fp8 SBUF with bias. Roughly half
                the evacuations go to ACT (Identity+bias) to balance DVE/ACT."""
                ps = pp.tile([128, S], f32, tag="proj")
                for g in range(3):
                    nc.tensor.matmul(
                        ps, w_sb[:, 2 * g:2 * g + 2, t * 128:(t + 1) * 128],
                        xT_sb[:, 2 * g:2 * g + 2, b * S:(b + 1) * S],
                        start=(g == 0), stop=(g == 2), perf_mode=PM.DoubleRow)
                if on_act:
                    nc.scalar.activation(dst[:, t, :], ps, AF.Identity,
                                         bias=b_sb[:, t:t + 1], scale=2.0 ** -5)
                else:
                    nc.vector.tensor_scalar(out=dst[:, t, :], in0=ps,
                                            scalar1=2.0 ** -5,
                                            scalar2=b_sb[:, t:t + 1],
                                            op0=OP.mult, op1=OP.add)

            def emit_v_group(b, vb, tl, cg):
                n = 512 if cg == 0 else 256
                ps = pp.tile([128, n], f32, tag="proj")
                tt = b * KT4 + tl
                for g in range(3):
                    nc.tensor.matmul(
                        ps, xT_sb[:, 2 * g:2 * g + 2, tt * 128:(tt + 1) * 128],
                        wv_sb[:, 2 * g:2 * g + 2, cg * 512:cg * 512 + n],
                        start=(g == 0), stop=(g == 2), perf_mode=PM.DoubleRow)
                nc.vector.tensor_scalar(out=vb[:, tl, cg * 8:cg * 8 + n // D, :],
                                        in0=ps, scalar1=2.0 ** -6, scalar2=None,
                                        op0=OP.mult)

            V_GROUPS = [(tl, cg) for tl in range(KT4) for cg in range(2)]
            V_SLICE = {0: [0], 1: [1], 2: [2], 3: [3], 4: [4, 5], 5: [6, 7]}

            def emit_proj_slice(b, pr, tiles):
                qb, kb, vb = tiles
                emit_qk_proj(b, pr, wq_sb, bq2_sb, qb)
                emit_qk_proj(b, pr, wk_sb, bk2_sb, kb, on_act=(pr % 2 == 1))
                for g in V_SLICE[pr]:
                    emit_v_group(b, vb, *V_GROUPS[g])

            def emit_o_chunk(ost, qt, split_pp=False):
                """O projection + residual + LN stats for one 128-token tile
                of batch ost['b'] (spread across the NEXT batch's pair loop)."""
                b, wt_sb, xrs = ost["b"], ost["wt"], ost["xrs"]
                y = yp.tile([128, H], f32, tag="y")
                # O psum lives in the wev ring (not pp): pp stays 4-deep per
                # pair (Q/K/V/st) so its 2 slots never gate the ACT evacs.
                # In the tail (split_pp) odd tiles use the pp ring instead so
                # two O projections are in flight.
                if split_pp:
                    op0 = pp.tile([128, 512], f32, tag="proj")
                    op1 = pp.tile([128, 256], f32, tag="proj")
                    pss = [op0, op1]
                else:
                    ops = wevp.tile([128, H], f32, tag="wev", bufs=1)
                    pss = [ops[:, 0:512], ops[:, 512:H]]
                for cg in range(2):
                    n = 512 if cg == 0 else 256
                    for j in range(PAIRS):
                        nc.tensor.matmul(
                            pss[cg],
                            wt_sb[:, 2 * j:2 * j + 2, qt * 128:(qt + 1) * 128],
                            wo_sb[:, 2 * j:2 * j + 2, cg * 512:cg * 512 + n],
                            start=(j == 0), stop=(j == PAIRS - 1),
                            perf_mode=PM.DoubleRow)
                if split_pp:
                    nc.vector.tensor_add(y[:, 0:512], xrs[qt][:, 0:512], pss[0])
                    nc.vector.tensor_add(y[:, 512:H], xrs[qt][:, 512:H], pss[1])
                else:
                    nc.vector.tensor_add(y, xrs[qt], ops)
                ost["ys"].append(y)
                stats = smalls.tile([128, 3, 6], f32, tag="st")
                for g in range(3):
                    nc.vector.bn_stats(stats[:, g, :], y[:, g * 256:(g + 1) * 256])
                nc.vector.bn_aggr(ost["mvb"][:, qt, :], stats)

            def emit_fin(ost, qts, alt_engine=False):
                """LN finalize (rstd + normalize + out DMAs) for tiles qts.
                rstd = exp(-0.5*ln(var+eps)); Ln and Exp share LUT set 6."""
                b, mvb, ys = ost["b"], ost["mvb"], ost["ys"]
                q0, nq = qts[0], len(qts)
                lnv = smalls.tile([128, nq], f32, tag="lnv")
                nc.scalar.activation(lnv, mvb[:, q0:q0 + nq, 1], AF.Ln,
                                     bias=eps_sb, scale=1.0)
                rstd = smalls.tile([128, nq], f32, tag="rstd")
                nc.scalar.activation(rstd, lnv, AF.Exp, bias=0.0, scale=-0.5)
                for i, qt in enumerate(qts):
                    o = lnp.tile([128, H], f32, tag="o")
                    eng = nc.vector if (alt_engine and qt % 2) else nc.gpsimd
                    eng.tensor_scalar(out=o, in0=ys[qt],
                                      scalar1=mvb[:, qt, 0:1],
                                      scalar2=rstd[:, i:i + 1],
                                      op0=OP.subtract, op1=OP.mult)
                    dge = nc.sync if alt_engine else nc.gpsimd
                    dge.dma_start(out=out_t[b * KT4 + qt], in_=o)

            def emit_scores_exp(b, pr, qb, kb):
                ex = expp.tile([128, KT4, 2 * S], fp8, tag="ex")
                for kt in range(KT4):
                    ps = scp.tile([128, 1024], f32, tag="sc")
                    for hh in range(2):
                        lo, hi = hh * 64, (hh + 1) * 64
                        nc.tensor.matmul(
                            ps[:, hh * 512:(hh + 1) * 512],
                            kb[lo:hi, pr, kt * 128:(kt + 1) * 128],
                            qb[lo:hi, pr, :],
                            start=True, stop=True)
                    nc.scalar.activation(ex[:, kt, :], ps, AF.Exp,
                                         bias=mask_sb[:, kt, b:b + 1],
                                         scale=2.0 ** -5)
                return ex

            def emit_s_chain(ex):
                """Transposed softmax sums sT[q%64, hh*8+qs] = sum_k ex/4 (tiny
                ones-rhs matmuls into the proj psum ring), partition-parallel
                reciprocal, then scatter+broadcast 1/s to a [64, 1024] tile."""
                st = pp.tile([64, 16], f32, tag="proj")
                for hh in range(2):
                    for qs in range(8):
                        c0 = hh * 512 + qs * 64
                        nc.tensor.matmul(
                            st[:, hh * 8 + qs:hh * 8 + qs + 1],
                            ex[:, 0:2, c0:c0 + 64], ones8_sb,
                            start=True, stop=False, perf_mode=PM.DoubleRow)
                        nc.tensor.matmul(
                            st[:, hh * 8 + qs:hh * 8 + qs + 1],
                            ex[:, 2:4, c0:c0 + 64], ones8_sb,
                            start=False, stop=True, perf_mode=PM.DoubleRow)
                str_sb = smalls.tile([64, 16], f32, tag="str", bufs=6)
                nc.vector.reciprocal(str_sb, st)
                # scatter 1/s into final column order dsr[c*64+l], so the
                # broadcast is one clean 2-dim contiguous DMA
                dsr = drp.tile([1, 1024], f32, tag="dsr")
                nc.sync.dma_start(
                    out=bass.AP(tensor=dsr.tensor, offset=dsr.offset,
                                ap=[[1, 64], [64, 16], [1, 1]]),
                    in_=str_sb)
                sbc = sbcp.tile([64, 1024], f32, tag="sbc")
                nc.sync.dma_start(out=sbc, in_=dsr.to_broadcast([64, 1024]))
                return sbc

            def emit_attnv_mult(vb, wt_sb, pr, ex, sbc):
                # both heads into one [64, 1024] psum; single-buffered is fine
                # because attnv(p+1) is emitted a full pair after mult(p)
                wev = wevp.tile([64, 1024], f32, tag="wev", bufs=1)
                for hh in range(2):
                    h = 2 * pr + hh
                    for g in range(2):
                        nc.tensor.matmul(
                            wev[:, hh * 512:(hh + 1) * 512],
                            vb[:, 2 * g:2 * g + 2, h, :],
                            ex[:, 2 * g:2 * g + 2, hh * 512:(hh + 1) * 512],
                            start=(g == 0), stop=(g == 1),
                            perf_mode=PM.DoubleRow)
                nc.vector.tensor_mul(wt_sb[:, 2 * pr:2 * pr + 2, :], wev, sbc)

            # prologue: batch 0 projections
            cur = alloc_qkv()
            for pr in range(PAIRS):
                emit_proj_slice(0, pr, cur)

            pending = None   # o_ln state of the previous batch
            st1 = None       # (vb, wt, pr, ex): awaiting s_chain (1 pair back)
            st2 = None       # (vb, wt, pr, ex, sbc): awaiting attnv (2 back)
            for b in range(B_LOC):
                qb, kb, vb = cur
                nxt = alloc_qkv() if b + 1 < B_LOC else None

                wt_sb = wtp.tile([64, NH, S], fp8, tag="wt")
                xrs = []
                # attn*V + normalize run one pair behind scores/exp (so the
                # 1/s scatter+broadcast DMA chain has a full pair of slack and
                # the DVE stream never head-of-line blocks on it); the
                # PREVIOUS batch's O-projection/LN spreads across pairs 0-4.
                for pr in range(PAIRS):
                    ex = emit_scores_exp(b, pr, qb, kb)
                    if nxt is not None:
                        emit_proj_slice(b + 1, pr, nxt)
                    if st2 is not None:
                        emit_attnv_mult(*st2)
                    if pending is not None:
                        if pr < KT4:
                            emit_o_chunk(pending, pr)
                        elif pr == KT4:
                            emit_fin(pending, range(KT4))
                            pending = None
                    if pr == 3:
                        for qt in range(KT4):
                            xr = lnp.tile([128, H], bf16, tag="xr", bufs=8)
                            nc.gpsimd.dma_start(out=xr, in_=xres_t[b * KT4 + qt])
                            xrs.append(xr)
                    st2 = (vb, wt_sb, pr, ex, emit_s_chain(ex))

                mvb = smalls.tile([128, KT4, 2], f32, tag="mvb")
                ost = {"b": b, "wt": wt_sb, "xrs": xrs, "ys": [], "mvb": mvb}
                if b < B_LOC - 1:
                    pending = ost
                else:
                    # tail: flush the in-flight pair, then per-qt finalize on
                    # alternating engines to shorten the drain
                    emit_attnv_mult(*st2)
                    st2 = None
                    for qt in range(KT4):
                        emit_o_chunk(ost, qt, split_pp=(qt % 2 == 1))
                        emit_fin(ost, [qt], alt_engine=True)
                cur = nxt

    nc.compile()
    return nc


def _get_nc():
    if "nc" not in _CACHE:
        _CACHE["nc"] = _build()
    return _CACHE["nc"]


def _prep_in_maps(inputs):
    x = np.asarray(inputs["x"], np.float32)
    mask = np.asarray(inputs["additive_attention_mask"], np.float32)
    Wq = np.asarray(inputs["Wq"], np.float32)
    Wk = np.asarray(inputs["Wk"], np.float32)
    Wv = np.asarray(inputs["Wv"], np.float32)
    Wo = np.asarray(inputs["Wo"], np.float32)
    bq = np.asarray(inputs["bq"], np.float32)
    bk = np.asarray(inputs["bk"], np.float32)
    bv = np.asarray(inputs["bv"], np.float32)
    bo = np.asarray(inputs["bo"], np.float32)

    wq8 = np.ascontiguousarray(Wq.T * 64.0).astype(FP8)
    wk8 = np.ascontiguousarray(Wk.T * 64.0).astype(FP8)
    wv8 = np.ascontiguousarray(Wv.T * 64.0).astype(FP8)
    wo8 = np.ascontiguousarray(
        (Wo.T * 64.0).reshape(NH, D, H).transpose(1, 0, 2)).astype(FP8)
    bq2 = (2.0 * bq).reshape(KT, 128).T
    bk2 = (2.0 * bk).reshape(KT, 128).T
    bo2 = bo + Wo @ bv  # attn weights sum to 1: bv passes through to O-proj

    shared = {"wq": wq8, "wk": wk8, "wv": wv8, "wo": wo8}
    in_maps = []
    for c in range(N_CORES):
        xs = x[c * B_LOC:(c + 1) * B_LOC].reshape(T, H)
        # mask as [128, kt, b]: token k = kt*128+p of batch b, minus ln4
        mc = (mask[c * B_LOC:(c + 1) * B_LOC, 0, 0, :] - np.log(4.0))
        mkb = mc.reshape(B_LOC, KT4, 128).transpose(2, 1, 0).reshape(128, KT4 * B_LOC)
        cst = np.concatenate([bq2, bk2, mkb], axis=1).astype(np.float32)
        in_maps.append({
            "xT": np.ascontiguousarray(xs.T).astype(FP8),
            "xres": (np.ascontiguousarray(xs + bo2[None, :]) * 256.0).astype(BF16),
            "cst": np.ascontiguousarray(cst),
            **shared,
        })
    return in_maps


def run(inputs, trace=False):
    """Returns (full_output, BassKernelResults)."""
    from concourse.bass_utils import run_bass_kernel_spmd

    nc = _get_nc()
    in_maps = _prep_in_maps(inputs)
    res = run_bass_kernel_spmd(nc, in_maps, core_ids=list(range(N_CORES)),
                               trace=trace)
    out = np.concatenate(
        [res.results[c]["out"].reshape(B_LOC, S, H) for c in range(N_CORES)], axis=0)
    ln_w = np.asarray(inputs["ln_w"], np.float32)
    ln_b = np.asarray(inputs["ln_b"], np.float32)
    out = out * ln_w[None, None, :] + ln_b[None, None, :]
    return np.ascontiguousarray(out.astype(np.float32)), res


def kernel(**inputs) -> np.ndarray:
    out, _ = run(inputs, trace=False)
    return out


# revision 51
# speedup vs baseline: 1.6845x; 1.0032x over previous
"""BertAttention (B=32, S=512, H=768, 12 heads) Bass/Tile kernel for 8 TRN2 cores.

Sharding: data-parallel over batch — 4 batches per NeuronCore. kernel() takes
the FULL inputs, slices/preps them on host, runs one SPMD NEFF on cores 0-7,
and reassembles the full (32, 512, 768) output.

All matmuls run in fp8 (e4m3); the deep contractions (Q/K/V projections,
attn*V over keys, O projection) use DoubleRow perf mode — two 128-deep
contraction subtiles per instruction at double rate. The scores matmul
contracts only d=64, so it runs as plain fp8 matmuls with the two heads of a
pair sharing the PE array at partition bases 0/64 (matmul operands may only
start at partition 0/32/64, which rules out a 4x32 d-folded DoubleRow):
  - exp() runs on ACT straight out of the scores psum (scale 1/32 folds the
    1/sqrt(64) and the fp8 q/k x2 scales; bias carries mask - ln4 so exp fits
    fp8 range). Softmax denominators are taken TRANSPOSED (s per q-token on
    partitions) by tiny ones-rhs matmuls off the same exp tiles, so the
    reciprocal is partition-parallel ([64,16] per pair), then broadcast to a
    [64, 1024] tile via a DRAM-bounce DMA; one DVE multiply per head both
    evacuates the attn*V psum and normalizes, writing the fp8 O-proj operand.
  - softmax bias bv is folded into bo on host (attn weights sum to 1), the
    residual is pre-scaled by 256 = product of all fp8 scale factors (ln is
    scale-invariant), and ln_w/ln_b are applied host-side on the output.
"""

import sys

for _p in ("/opt/trn_rl_repo",):
    if _p not in sys.path:
        sys.path.insert(0, _p)

import numpy as np
import ml_dtypes

FP8 = ml_dtypes.float8_e4m3
BF16 = ml_dtypes.bfloat16

N_CORES = 8
B_LOC = 4            # batches per core
S = 512              # sequence length
T = B_LOC * S        # tokens per core
H = 768              # hidden
NH = 12              # heads
D = 64               # head size
KT = 6               # 128-wide hidden tiles
PAIRS = NH // 2      # head pairs (6)
KT4 = S // 128       # 128-wide key-token tiles per batch (4)

_CACHE = {}


def _build():
    import concourse.bacc as bacc
    import concourse.tile as tile
    from concourse import mybir
    import concourse.bass as bass

    f32 = mybir.dt.float32
    bf16 = mybir.dt.bfloat16
    fp8 = mybir.dt.float8e4
    AF = mybir.ActivationFunctionType
    OP = mybir.AluOpType
    PM = mybir.MatmulPerfMode

    nc = bacc.Bacc("TRN2", target_bir_lowering=False, debug=False,
                   enable_asserts=False, num_devices=N_CORES)

    xT_d = nc.dram_tensor("xT", [H, T], fp8, kind="ExternalInput").ap()
    xres_d = nc.dram_tensor("xres", [T, H], bf16, kind="ExternalInput").ap()
    wq_d = nc.dram_tensor("wq", [H, H], fp8, kind="ExternalInput").ap()
    wk_d = nc.dram_tensor("wk", [H, H], fp8, kind="ExternalInput").ap()
    wv_d = nc.dram_tensor("wv", [H, H], fp8, kind="ExternalInput").ap()
    wo_d = nc.dram_tensor("wo", [D, NH, H], fp8, kind="ExternalInput").ap()
    cst_d = nc.dram_tensor("cst", [128, 2 * KT + KT4 * B_LOC], f32,
                           kind="ExternalInput").ap()
    out_d = nc.dram_tensor("out", [T, H], f32, kind="ExternalOutput").ap()

    xres_t = xres_d.rearrange("(tt p) h -> tt p h", p=128)
    out_t = out_d.rearrange("(tt p) h -> tt p h", p=128)

    with tile.TileContext(nc) as tc:
        with tc.tile_pool(name="persist", bufs=1) as persist, \
             tc.tile_pool(name="qkv", bufs=2) as qkv, \
             tc.tile_pool(name="expp", bufs=5) as expp, \
             tc.tile_pool(name="wtp", bufs=2) as wtp, \
             tc.tile_pool(name="sbcp", bufs=6) as sbcp, \
             tc.tile_pool(name="smalls", bufs=4) as smalls, \
             tc.tile_pool(name="lnp", bufs=3) as lnp, \
             tc.tile_pool(name="yp", bufs=6) as yp, \
             tc.tile_pool(name="drp", bufs=12, space="DRAM") as drp, \
             tc.tile_pool(name="pp", bufs=2, space="PSUM") as pp, \
             tc.tile_pool(name="scp", bufs=2, space="PSUM") as scp, \
             tc.tile_pool(name="wevp", bufs=2, space="PSUM") as wevp:
            # ---- persistent tensors ----
            xT_sb = persist.tile([128, KT, T], fp8)       # [p, kt, tok]
            wq_sb = persist.tile([128, KT, H], fp8)
            wk_sb = persist.tile([128, KT, H], fp8)
            wv_sb = persist.tile([128, KT, H], fp8)
            wo_sb = persist.tile([D, NH, H], fp8)         # [d, head, hid_out]
            cst_sb = persist.tile([128, 2 * KT + KT4 * B_LOC], f32)
            bq2_sb = cst_sb[:, 0:KT]
            bk2_sb = cst_sb[:, KT:2 * KT]
            mask_sb = cst_sb[:, 2 * KT:].rearrange("p (kt b) -> p kt b", b=B_LOC)
            eps_sb = persist.tile([128, 1], f32)
            ones8_sb = persist.tile([128, 2, 1], fp8)     # 0.25: softmax-sum rhs

            xT_t = xT_d.rearrange("(kt p) t -> p kt t", p=128)
            wq_t = wq_d.rearrange("(kt p) j -> p kt j", p=128)
            wk_t = wk_d.rearrange("(kt p) j -> p kt j", p=128)
            wv_t = wv_d.rearrange("(kt p) j -> p kt j", p=128)
            # ordered so batch-0 pair-0 operands land first: tiny tensors,
            # x(b0), then per-jt column chunks of Wq/Wk interleaved with Wv
            nc.sync.dma_start(out=cst_sb, in_=cst_d)
            nc.sync.dma_start(out=xT_sb[:, :, 0:S], in_=xT_t[:, :, 0:S])
            # Wq/Wk split in column halves: pairs 0-2 can start ~2.5us sooner
            nc.sync.dma_start(out=wq_sb[:, :, 0:384], in_=wq_t[:, :, 0:384])
            nc.sync.dma_start(out=wk_sb[:, :, 0:384], in_=wk_t[:, :, 0:384])
            nc.sync.dma_start(out=wq_sb[:, :, 384:H], in_=wq_t[:, :, 384:H])
            nc.sync.dma_start(out=wk_sb[:, :, 384:H], in_=wk_t[:, :, 384:H])
            nc.sync.dma_start(out=wv_sb, in_=wv_t)
            for bb in range(1, B_LOC):
                nc.sync.dma_start(out=xT_sb[:, :, bb * S:(bb + 1) * S],
                                  in_=xT_t[:, :, bb * S:(bb + 1) * S])
            nc.sync.dma_start(out=wo_sb, in_=wo_d)
            nc.vector.memset(eps_sb, 1e-12)
            nc.vector.memset(ones8_sb, 0.25)
            # Pre-load ACT LUT set 6 (natural_log_exp_and_others): holds Exp
            # and Ln, the only ACT functions used, so no table reloads occur.
            _tables = list(__import__("concourse.hw_specs", fromlist=["x"])
                           .get_activation_tables(nc.m.arch))
            _set6 = _tables.index("natural_log_exp_and_others")
            nc.scalar.add_instruction(mybir.InstLoadActFuncSet(
                name=nc.get_next_instruction_name(), ins=[], outs=[],
                act_func_set_id=_set6))

            # ---- per-batch emission helpers (software-pipelined below) ----
            def alloc_qkv():
                qb = qkv.tile([128, PAIRS, S], fp8, tag="qb")  # [p, jt, tok]
                kb = qkv.tile([128, PAIRS, S], fp8, tag="kb")
                vb = qkv.tile([128, KT4, NH, D], fp8, tag="vb")  # [ktok, tl, head, d]
                return qb, kb, vb

            def emit_qk_proj(b, t, w_sb, b_sb, dst, on_act=False):
                """One Q/K psum tile jt=t -> fp8 SBUF with bias. Roughly half
                the evacuations go to ACT (Identity+bias) to balance DVE/ACT."""
                ps = pp.tile([128, S], f32, tag="proj")
                for g in range(3):
                    nc.tensor.matmul(
                        ps, w_sb[:, 2 * g:2 * g + 2, t * 128:(t + 1) * 128],
                        xT_sb[:, 2 * g:2 * g + 2, b * S:(b + 1) * S],
                        start=(g == 0), stop=(g == 2), perf_mode=PM.DoubleRow)
                if on_act:
                    nc.scalar.activation(dst[:, t, :], ps, AF.Identity,
                                         bias=b_sb[:, t:t + 1], scale=2.0 ** -5)
                else:
                    nc.vector.tensor_scalar(out=dst[:, t, :], in0=ps,
                                            scalar1=2.0 ** -5,
                                            scalar2=b_sb[:, t:t + 1],
                                            op0=OP.mult, op1=OP.add)

            def emit_v_group(b, vb, tl, cg):
                n = 512 if cg == 0 else 256
                ps = pp.tile([128, n], f32, tag="proj")
                tt = b * KT4 + tl
                for g in range(3):
                    nc.tensor.matmul(
                        ps, xT_sb[:, 2 * g:2 * g + 2, tt * 128:(tt + 1) * 128],
                        wv_sb[:, 2 * g:2 * g + 2, cg * 512:cg * 512 + n],
                        start=(g == 0), stop=(g == 2), perf_mode=PM.DoubleRow)
                nc.vector.tensor_scalar(out=vb[:, tl, cg * 8:cg * 8 + n // D, :],
                                        in0=ps, scalar1=2.0 ** -6, scalar2=None,
                                        op0=OP.mult)

            V_GROUPS = [(tl, cg) for tl in range(KT4) for cg in range(2)]
            V_SLICE = {0: [0], 1: [1], 2: [2], 3: [3], 4: [4, 5], 5: [6, 7]}

            def emit_proj_slice(b, pr, tiles):
                qb, kb, vb = tiles
                emit_qk_proj(b, pr, wq_sb, bq2_sb, qb)
                emit_qk_proj(b, pr, wk_sb, bk2_sb, kb, on_act=(pr % 2 == 1))
                for g in V_SLICE[pr]:
                    emit_v_group(b, vb, *V_GROUPS[g])

            def emit_o_chunk(ost, qt, split_pp=False):
                """O projection + residual + LN stats for one 128-token tile
                of batch ost['b'] (spread across the NEXT batch's pair loop)."""
                b, wt_sb, xrs = ost["b"], ost["wt"], ost["xrs"]
                y = yp.tile([128, H], f32, tag="y")
                # O psum lives in the wev ring (not pp): pp stays 4-deep per
                # pair (Q/K/V/st) so its 2 slots never gate the ACT evacs.
                # In the tail (split_pp) odd tiles use the pp ring instead so
                # two O projections are in flight.
                if split_pp:
                    op0 = pp.tile([128, 512], f32, tag="proj")
                    op1 = pp.tile([128, 256], f32, tag="proj")
                    pss = [op0, op1]
                else:
                    ops = wevp.tile([128, H], f32, tag="wev", bufs=1)
                    pss = [ops[:, 0:512], ops[:, 512:H]]
                for cg in range(2):
                    n = 512 if cg == 0 else 256
                    for j in range(PAIRS):
                        nc.tensor.matmul(
                            pss[cg],
                            wt_sb[:, 2 * j:2 * j + 2, qt * 128:(qt + 1) * 128],
                            wo_sb[:, 2 * j:2 * j + 2, cg * 512:cg * 512 + n],
                            start=(j == 0), stop=(j == PAIRS - 1),
                            perf_mode=PM.DoubleRow)
                if split_pp:
                    nc.vector.tensor_add(y[:, 0:512], xrs[qt][:, 0:512], pss[0])
                    nc.vector.tensor_add(y[:, 512:H], xrs[qt][:, 512:H], pss[1])
                else:
                    nc.vector.tensor_add(y, xrs[qt], ops)
                ost["ys"].append(y)
                stats = smalls.tile([128, 2, 6], f32, tag="st")
                for g in range(2):
                    nc.vector.bn_stats(stats[:, g, :], y[:, g * 384:(g + 1) * 384])
                nc.vector.bn_aggr(ost["mvb"][:, qt, :], stats)

            def emit_fin(ost, qts, alt_engine=False):
                """LN finalize (rstd + normalize + out DMAs) for tiles qts.
                rstd = exp(-0.5*ln(var+eps)); Ln and Exp share LUT set 6."""
                b, mvb, ys = ost["b"], ost["mvb"], ost["ys"]
                q0, nq = qts[0], len(qts)
                lnv = smalls.tile([128, nq], f32, tag="lnv")
                nc.scalar.activation(lnv, mvb[:, q0:q0 + nq, 1], AF.Ln,
                                     bias=eps_sb, scale=1.0)
                rstd = smalls.tile([128, nq], f32, tag="rstd")
                nc.scalar.activation(rstd, lnv, AF.Exp, bias=0.0, scale=-0.5)
                for i, qt in enumerate(qts):
                    o = lnp.tile([128, H], f32, tag="o")
                    eng = nc.vector if (alt_engine and qt % 2) else nc.gpsimd
                    eng.tensor_scalar(out=o, in0=ys[qt],
                                      scalar1=mvb[:, qt, 0:1],
                                      scalar2=rstd[:, i:i + 1],
                                      op0=OP.subtract, op1=OP.mult)
                    dge = nc.sync if alt_engine else nc.gpsimd
                    dge.dma_start(out=out_t[b * KT4 + qt], in_=o)

            def emit_scores_exp(b, pr, qb, kb):
                ex = expp.tile([128, KT4, 2 * S], fp8, tag="ex")
                for kt in range(KT4):
                    ps = scp.tile([128, 1024], f32, tag="sc")
                    for hh in range(2):
                        lo, hi = hh * 64, (hh + 1) * 64
                        nc.tensor.matmul(
                            ps[:, hh * 512:(hh + 1) * 512],
                            kb[lo:hi, pr, kt * 128:(kt + 1) * 128],
                            qb[lo:hi, pr, :],
                            start=True, stop=True)
                    nc.scalar.activation(ex[:, kt, :], ps, AF.Exp,
                                         bias=mask_sb[:, kt, b:b + 1],
                                         scale=2.0 ** -5)
                return ex

            def emit_s_chain(ex):
                """Transposed softmax sums sT[q%64, hh*8+qs] = sum_k ex/4 (tiny
                ones-rhs matmuls into the proj psum ring), partition-parallel
                reciprocal, then scatter+broadcast 1/s to a [64, 1024] tile."""
                st = pp.tile([64, 16], f32, tag="proj")
                for hh in range(2):
                    for qs in range(8):
                        c0 = hh * 512 + qs * 64
                        nc.tensor.matmul(
                            st[:, hh * 8 + qs:hh * 8 + qs + 1],
                            ex[:, 0:2, c0:c0 + 64], ones8_sb,
                            start=True, stop=False, perf_mode=PM.DoubleRow)
                        nc.tensor.matmul(
                            st[:, hh * 8 + qs:hh * 8 + qs + 1],
                            ex[:, 2:4, c0:c0 + 64], ones8_sb,
                            start=False, stop=True, perf_mode=PM.DoubleRow)
                str_sb = smalls.tile([64, 16], f32, tag="str", bufs=6)
                nc.vector.reciprocal(str_sb, st)
                # scatter 1/s into final column order dsr[c*64+l], so the
                # broadcast is one clean 2-dim contiguous DMA
                dsr = drp.tile([1, 1024], f32, tag="dsr")
                nc.sync.dma_start(
                    out=bass.AP(tensor=dsr.tensor, offset=dsr.offset,
                                ap=[[1, 64], [64, 16], [1, 1]]),
                    in_=str_sb)
                sbc = sbcp.tile([64, 1024], f32, tag="sbc")
                nc.sync.dma_start(out=sbc, in_=dsr.to_broadcast([64, 1024]))
                return sbc

            def emit_attnv_mult(vb, wt_sb, pr, ex, sbc):
                # both heads into one [64, 1024] psum; single-buffered is fine
                # because attnv(p+1) is emitted a full pair after mult(p)
                wev = wevp.tile([64, 1024], f32, tag="wev", bufs=1)
                for hh in range(2):
                    h = 2 * pr + hh
                    for g in range(2):
                        nc.tensor.matmul(
                            wev[:, hh * 512:(hh + 1) * 512],
                            vb[:, 2 * g:2 * g + 2, h, :],
                            ex[:, 2 * g:2 * g + 2, hh * 512:(hh + 1) * 512],
                            start=(g == 0), stop=(g == 1),
                            perf_mode=PM.DoubleRow)
                nc.vector.tensor_mul(wt_sb[:, 2 * pr:2 * pr + 2, :], wev, sbc)

            # prologue: batch 0 projections
            cur = alloc_qkv()
            for pr in range(PAIRS):
                emit_proj_slice(0, pr, cur)

            pending = None   # o_ln state of the previous batch
            st1 = None       # (vb, wt, pr, ex): awaiting s_chain (1 pair back)
            st2 = None       # (vb, wt, pr, ex, sbc): awaiting attnv (2 back)
            for b in range(B_LOC):
                qb, kb, vb = cur
                nxt = alloc_qkv() if b + 1 < B_LOC else None

                wt_sb = wtp.tile([64, NH, S], fp8, tag="wt")
                xrs = []
                # attn*V + normalize run one pair behind scores/exp (so the
                # 1/s scatter+broadcast DMA chain has a full pair of slack and
                # the DVE stream never head-of-line blocks on it); the
                # PREVIOUS batch's O-projection/LN spreads across pairs 0-4.
                for pr in range(PAIRS):
                    ex = emit_scores_exp(b, pr, qb, kb)
                    if nxt is not None:
                        emit_proj_slice(b + 1, pr, nxt)
                    if st2 is not None:
                        emit_attnv_mult(*st2)
                    if pending is not None:
                        if pr < KT4:
                            emit_o_chunk(pending, pr)
                        elif pr == KT4:
                            emit_fin(pending, range(KT4))
                            pending = None
                    if pr == 3:
                        for qt in range(KT4):
                            xr = lnp.tile([128, H], bf16, tag="xr", bufs=8)
                            nc.gpsimd.dma_start(out=xr, in_=xres_t[b * KT4 + qt])
                            xrs.append(xr)
                    st2 = (vb, wt_sb, pr, ex, emit_s_chain(ex))
                    if b == B_LOC - 1:
                        # last batch: flush same-pair (DVE has slack to absorb
                        # the broadcast wait), so the drain starts earlier
                        emit_attnv_mult(*st2)
                        st2 = None

                mvb = smalls.tile([128, KT4, 2], f32, tag="mvb")
                ost = {"b": b, "wt": wt_sb, "xrs": xrs, "ys": [], "mvb": mvb}
                if b < B_LOC - 1:
                    pending = ost
                else:
                    # tail: per-qt finalize on alternating engines/psum rings
                    for qt in range(KT4):
                        emit_o_chunk(ost, qt, split_pp=(qt % 2 == 1))
                        emit_fin(ost, [qt], alt_engine=True)
                cur = nxt

    nc.compile()
    return nc


def _get_nc():
    if "nc" not in _CACHE:
        _CACHE["nc"] = _build()
    return _CACHE["nc"]


def _prep_in_maps(inputs):
    x = np.asarray(inputs["x"], np.float32)
    mask = np.asarray(inputs["additive_attention_mask"], np.float32)
    Wq = np.asarray(inputs["Wq"], np.float32)
    Wk = np.asarray(inputs["Wk"], np.float32)
    Wv = np.asarray(inputs["Wv"], np.float32)
    Wo = np.asarray(inputs["Wo"], np.float32)
    bq = np.asarray(inputs["bq"], np.float32)
    bk = np.asarray(inputs["bk"], np.float32)
    bv = np.asarray(inputs["bv"], np.float32)
    bo = np.asarray(inputs["bo"], np.float32)

    wq8 = np.ascontiguousarray(Wq.T * 64.0).astype(FP8)
    wk8 = np.ascontiguousarray(Wk.T * 64.0).astype(FP8)
    wv8 = np.ascontiguousarray(Wv.T * 64.0).astype(FP8)
    wo8 = np.ascontiguousarray(
        (Wo.T * 64.0).reshape(NH, D, H).transpose(1, 0, 2)).astype(FP8)
    bq2 = (2.0 * bq).reshape(KT, 128).T
    bk2 = (2.0 * bk).reshape(KT, 128).T
    bo2 = bo + Wo @ bv  # attn weights sum to 1: bv passes through to O-proj

    shared = {"wq": wq8, "wk": wk8, "wv": wv8, "wo": wo8}
    in_maps = []
    for c in range(N_CORES):
        xs = x[c * B_LOC:(c + 1) * B_LOC].reshape(T, H)
        # mask as [128, kt, b]: token k = kt*128+p of batch b, minus ln4
        mc = (mask[c * B_LOC:(c + 1) * B_LOC, 0, 0, :] - np.log(4.0))
        mkb = mc.reshape(B_LOC, KT4, 128).transpose(2, 1, 0).reshape(128, KT4 * B_LOC)
        cst = np.concatenate([bq2, bk2, mkb], axis=1).astype(np.float32)
        in_maps.append({
            "xT": np.ascontiguousarray(xs.T).astype(FP8),
            "xres": (np.ascontiguousarray(xs + bo2[None, :]) * 256.0).astype(BF16),
            "cst": np.ascontiguousarray(cst),
            **shared,
        })
    return in_maps


def run(inputs, trace=False):
    """Returns (full_output, BassKernelResults)."""
    from concourse.bass_utils import run_bass_kernel_spmd

    nc = _get_nc()
    in_maps = _prep_in_maps(inputs)
    res = run_bass_kernel_spmd(nc, in_maps, core_ids=list(range(N_CORES)),
                               trace=trace)
    out = np.concatenate(
        [res.results[c]["out"].reshape(B_LOC, S, H) for c in range(N_CORES)], axis=0)
    ln_w = np.asarray(inputs["ln_w"], np.float32)
    ln_b = np.asarray(inputs["ln_b"], np.float32)
    out = out * ln_w[None, None, :] + ln_b[None, None, :]
    return np.ascontiguousarray(out.astype(np.float32)), res


def kernel(**inputs) -> np.ndarray:
    out, _ = run(inputs, trace=False)
    return out
